# revision 1
# baseline (speedup 1.0000x reference)
"""Trainium2 Bass kernel: fused QKV + RoPE + causal/windowed GQA attention + output proj.

Sharding: tensor-parallel by head across 8 cores. Core c owns Q-heads
4c..4c+3 and KV-group c (matching repeat_interleave grouping), plus the
512 w_o columns for those heads. Each core computes a full-shape partial
of the final output (contraction over its 512 attention-output dims);
the host sums the 8 partials. No device collectives.

Dataflow is in transposed-activation space so every matmul contracts on
the partition dim; matmul operands are bf16 (full PE rate), all
accumulation/softmax math is fp32 in PSUM:
  P1: qkvT[e, tok] = w_qkvT^T @ xT           (xT pre-transposed on host)
  P2: ST[k, q] = kT^T @ qT  -> exp -> PV and row-sum both as matmuls
      (softmax normalization via reciprocal + partition_broadcast)
  P3: out_partial[tok, e] = outT^T @ w_oT    (outT kept SBUF-resident)

RoPE is applied on interleaved even/odd pairs via a DVE stream_shuffle
pair swap and a sign-folded sin table.
"""

import math
import sys
from contextlib import ExitStack

import numpy as np

sys.path.insert(0, "/opt/trn_rl_repo")

import ml_dtypes

BF16NP = ml_dtypes.bfloat16

import concourse.bass as bass
import concourse.mybir as mybir
import concourse.tile as tile
from concourse import bacc

F32 = mybir.dt.float32
F32R = mybir.dt.float32r
BF16 = mybir.dt.bfloat16

B, T, D = 2, 2048, 4096
H, G, HD = 32, 8, 128
THETA = 10000.0
NCORES = 8
HL = H // NCORES            # 4 local q heads
TOK = B * T                 # 4096
QROWS = HL * HD             # 512 local q rows
E = QROWS + 2 * HD          # 768 local qkv rows
SCALE = 1.0 / math.sqrt(HD)

TOKG = 256                  # P1 token-group width
NTOKG = TOK // TOKG
NDC = D // 128              # 32 contraction chunks
NE = E // 128               # 6 qkv row chunks
QG = 512                    # P2 query-group width (within batch)
NQG = T // QG               # 4
NKC = T // 128              # 16 key chunks per batch
MASK_NEG = -1.0e30


def _mask_plan(window: int):
    """Per (qgroup, kchunk): 'skip', 'full', or a mask-key (delta-based)."""
    plan = {}
    keys = {}
    for g in range(NQG):
        for kc in range(NKC):
            i_min, i_max = QG * g, QG * g + QG - 1
            j_min, j_max = 128 * kc, 128 * kc + 127
            if j_min > i_max or (i_min - j_max) >= window:
                plan[(g, kc)] = ("skip", None)
            elif j_max <= i_min and (i_max - j_min) < window:
                plan[(g, kc)] = ("full", None)
            else:
                key = QG * g - 128 * kc
                if key not in keys:
                    keys[key] = len(keys)
                plan[(g, kc)] = ("mask", keys[key])
    return plan, keys


def _build_masks(window: int, keys: dict) -> np.ndarray:
    n = max(1, len(keys))
    m = np.zeros((n, 128, QG), dtype=np.float32)  # cast to bf16 in kernel()
    for key, idx in keys.items():
        # i = key + 128*kc ... i - j = key + qq - kk
        qq = np.arange(QG)[None, :]
        kk = np.arange(128)[:, None]
        diff = key + qq - kk          # i - j
        vis = (diff >= 0) & (diff < window)
        m[idx] = np.where(vis, 1.0, 0.0)
    return m


PAIRSWAP = [i ^ 1 for i in range(32)]


def _rope_ops(nc, pool, dst, src, cos_ap, sin_ap):
    """Interleaved-pair RoPE: dst = src*cos + pairswap(src)*signed_sin.

    cos_ap rows (2i, 2i+1) hold cos_i; sin_ap rows hold (-sin_i, +sin_i).
    src may alias dst (in-place).
    """
    W = dst.shape[-1]
    sw = pool.tile([128, W], BF16, tag="rope_sw")
    tmp = pool.tile([128, W], BF16, tag="rope_tmp")
    qc = pool.tile([128, W], BF16, tag="rope_qc")
    mult = mybir.AluOpType.mult
    nc.vector.stream_shuffle(sw, src, PAIRSWAP)
    nc.vector.tensor_tensor(tmp, sw, sin_ap, mult)
    nc.vector.tensor_tensor(qc, src, cos_ap, mult)
    nc.vector.tensor_tensor(dst, qc, tmp, mybir.AluOpType.add)


class _PhaseStop(Exception):
    pass


def build_nc(window: int, phases=(1, 2, 3)):
    plan, keys = _mask_plan(window)
    nmask = max(1, len(keys))

    nc = bacc.Bacc()
    xT_d = nc.dram_tensor("xT", [D, TOK], BF16, kind="ExternalInput")
    wqkvT_d = nc.dram_tensor("wqkvT", [D, E], BF16, kind="ExternalInput")
    woT_d = nc.dram_tensor("woT", [QROWS, D], BF16, kind="ExternalInput")
    cos_d = nc.dram_tensor("cosH", [128, T], BF16, kind="ExternalInput")
    sin_d = nc.dram_tensor("sinH", [128, T], BF16, kind="ExternalInput")
    masks_d = nc.dram_tensor("masks", [nmask, 128, QG], BF16, kind="ExternalInput")
    ident_d = nc.dram_tensor("ident", [128, 128], BF16, kind="ExternalInput")
    out_d = nc.dram_tensor("out", [TOK, D], F32, kind="ExternalOutput")

    with ExitStack() as octx:
        tc = octx.enter_context(tile.TileContext(nc))
        qkvp = octx.enter_context(tc.tile_pool(name="qkvT", bufs=1))
        qkvT_sb = [qkvp.tile([128, TOK], BF16, tag=f"qkv{e}", name=f"qkv{e}")
                   for e in range(NE)]

        # ---------------- P1: qkvT = w^T @ xT ----------------
        if 1 in phases:
         with ExitStack() as ctx:
            wpool = ctx.enter_context(tc.tile_pool(name="w1", bufs=1))
            xpool = ctx.enter_context(tc.tile_pool(name="x1", bufs=3))
            ppool = ctx.enter_context(tc.tile_pool(name="ps1", bufs=6, space="PSUM"))

            wsb = wpool.tile([128, NDC, E], BF16)
            wq_r = wqkvT_d[:].rearrange("(dc p) e -> p dc e", p=128)
            for dc in range(NDC):
                nc.sync.dma_start(out=wsb[:, dc, :], in_=wq_r[:, dc, :])
            for g in range(NTOKG):
                xsb = xpool.tile([128, NDC, TOKG], BF16, tag="xslab")
                x_r = xT_d[:, g * TOKG:(g + 1) * TOKG].rearrange(
                    "(dc p) t -> p dc t", p=128)
                for dq in range(4):
                    nc.sync.dma_start(out=xsb[:, dq * 8:(dq + 1) * 8, :],
                                      in_=x_r[:, dq * 8:(dq + 1) * 8, :])
                for e in range(NE):
                    ps = ppool.tile([128, TOKG], F32, tag="p1")
                    for dc in range(NDC):
                        nc.tensor.matmul(
                            ps,
                            lhsT=wsb[:, dc, e * 128:(e + 1) * 128],
                            rhs=xsb[:, dc, :],
                            start=(dc == 0), stop=(dc == NDC - 1))
                    # fold softmax 1/sqrt(HD) into q rows; evict into the
                    # SBUF-resident qkvT directly
                    nc.scalar.mul(
                        qkvT_sb[e][:, g * TOKG:(g + 1) * TOKG], ps,
                        SCALE if e < HL else 1.0)

        # ---------------- P2: attention ----------------
        if 2 in phases:
            # outT survives P2 -> P3: allocate after P1's pools are released.
            opool = octx.enter_context(tc.tile_pool(name="outT", bufs=1))
            outT = [opool.tile([128, TOK], BF16, tag=f"outT{i}", name=f"outT{i}")
                    for i in range(HL)]
            p2ctx = ExitStack()
            kpool = p2ctx.enter_context(tc.tile_pool(name="kv", bufs=1))
            ksb = qkvT_sb[HL]
            vsb = kpool.tile([128, TOK // 128, 128], BF16, tag="v")
            cos_sb = kpool.tile([128, T], BF16, tag="cos")
            sin_sb = kpool.tile([128, T], BF16, tag="sin")
            ones_sb = kpool.tile([128, 1], BF16, tag="ones")
            mask_sb = kpool.tile([128, nmask, QG], BF16, tag="masks")

            nc.sync.dma_start(out=cos_sb, in_=cos_d[:])
            nc.sync.dma_start(out=sin_sb, in_=sin_d[:])
            nc.sync.dma_start(
                out=mask_sb, in_=masks_d[:].rearrange("n p q -> p n q"))
            nc.vector.memset(ones_sb, 1.0)

            with ExitStack() as ctx:
                sc0 = ctx.enter_context(tc.tile_pool(name="p2a", bufs=1))
                pt0 = ctx.enter_context(tc.tile_pool(name="p2aps", bufs=2, space="PSUM"))
                ident = sc0.tile([128, 128], BF16, tag="ident")
                nc.sync.dma_start(out=ident, in_=ident_d[:])
                vT = qkvT_sb[HL + 1]
                for tc32 in range(TOK // 128):
                    pst = pt0.tile([128, 128], BF16, tag="tr")
                    nc.tensor.transpose(
                        pst, vT[:, tc32 * 128:(tc32 + 1) * 128], ident)
                    nc.scalar.copy(vsb[:, tc32, :], pst)
                # RoPE on k (per batch)
                for b in range(B):
                    kslice = ksb[:, b * T:(b + 1) * T]
                    _rope_ops(nc, sc0, kslice, kslice, cos_sb, sin_sb)

            # P2 attention interleaved with P3 (output projection): after the 4
            # head-instances of a (batch, q-group) window finish, that window's
            # outT columns are final, so its P3 tiles are emitted immediately —
            # the Tile scheduler uses them to fill PE gaps in later P2 windows.
            with ExitStack() as ctx:
                qpool = ctx.enter_context(tc.tile_pool(name="q2", bufs=4))
                spool = ctx.enter_context(tc.tile_pool(name="sc2", bufs=4))
                estp = ctx.enter_context(tc.tile_pool(name="est", bufs=6))
                wpool = ctx.enter_context(tc.tile_pool(name="wo", bufs=1))
                panp = ctx.enter_context(tc.tile_pool(name="pan", bufs=2))
                stps = ctx.enter_context(tc.tile_pool(name="stps", bufs=3, space="PSUM"))
                rps = ctx.enter_context(tc.tile_pool(name="rps", bufs=1, space="PSUM"))
                ops = ctx.enter_context(tc.tile_pool(name="ops", bufs=2, space="PSUM"))
                pps = ctx.enter_context(tc.tile_pool(name="ps3", bufs=2, space="PSUM"))

                wo = []
                for dc in range(HL):
                    w = wpool.tile([128, D], BF16, tag=f"wo{dc}", name=f"wo{dc}")
                    nc.sync.dma_start(
                        out=w, in_=woT_d[dc * 128:(dc + 1) * 128, :])
                    wo.append(w)

                for b in range(B):
                    for g in range(NQG):
                        for hh in range(HL):
                            qsb = qpool.tile([128, QG], BF16, tag="q")
                            _rope_ops(nc, qpool, qsb,
                                      qkvT_sb[hh][:, b * T + g * QG:
                                                  b * T + (g + 1) * QG],
                                      cos_sb[:, g * QG:(g + 1) * QG],
                                      sin_sb[:, g * QG:(g + 1) * QG])
                            vis = [(kc, plan[(g, kc)]) for kc in range(NKC)
                                   if plan[(g, kc)][0] != "skip"]
                            r_ps = rps.tile([1, QG], F32, tag="r")
                            o_ps = ops.tile([128, QG], F32, tag="o")
                            for idx, (kc, (kind, mid)) in enumerate(vis):
                                # visible query subrange of this key chunk:
                                # qq >= -aoff (causal), qq < w - aoff + 127
                                aoff = QG * g - 128 * kc
                                qlo = max(0, -aoff)
                                qhi = min(QG, window - aoff + 127)
                                qsl = slice(qlo, qhi)
                                st = stps.tile([128, QG], F32, tag="st")
                                nc.tensor.matmul(
                                    st[:, qsl],
                                    lhsT=ksb[:, b * T + kc * 128:
                                             b * T + (kc + 1) * 128],
                                    rhs=qsb[:, qsl],
                                    start=True, stop=True)
                                est = estp.tile([128, QG], BF16, tag="est")
                                nc.scalar.activation(
                                    est[:, qsl], st[:, qsl],
                                    mybir.ActivationFunctionType.Exp)
                                if kind == "mask":
                                    nc.vector.tensor_tensor(
                                        est[:, qsl], est[:, qsl],
                                        mask_sb[:, mid, qsl],
                                        mybir.AluOpType.mult)
                                last = idx == len(vis) - 1
                                nc.tensor.matmul(
                                    r_ps[:, qsl], lhsT=ones_sb,
                                    rhs=est[:, qsl],
                                    start=(idx == 0), stop=last)
                                nc.tensor.matmul(
                                    o_ps[:, qsl],
                                    lhsT=vsb[:, b * NKC + kc, :],
                                    rhs=est[:, qsl],
                                    start=(idx == 0), stop=last)
                            rrec = spool.tile([1, QG], F32, tag="rrec")
                            nc.vector.reciprocal(rrec, r_ps)
                            rb = spool.tile([128, QG], F32, tag="rb")
                            nc.gpsimd.partition_broadcast(rb, rrec)
                            nc.vector.tensor_tensor(
                                outT[hh][:, b * T + g * QG: b * T + (g + 1) * QG],
                                o_ps, rb, mybir.AluOpType.mult)

                        # P3 for this window's 4 token chunks
                        for tloc in range(QG // 128):
                            tch = (b * T + g * QG) // 128 + tloc
                            panel = panp.tile([128, D], F32, tag="panel")
                            for et in range(D // 512):
                                ps = pps.tile([128, 512], F32, tag="p3")
                                for dc in range(HL):
                                    nc.tensor.matmul(
                                        ps,
                                        lhsT=outT[dc][:,
                                                      tch * 128:(tch + 1) * 128],
                                        rhs=wo[dc][:, et * 512:(et + 1) * 512],
                                        start=(dc == 0), stop=(dc == HL - 1))
                                nc.scalar.copy(
                                    panel[:, et * 512:(et + 1) * 512], ps)
                            nc.sync.dma_start(
                                out=out_d[tch * 128:(tch + 1) * 128, :], in_=panel)

            p2ctx.close()

    nc.finalize()
    return nc, nmask


_CACHE = {}


def _get_nc(window: int):
    if window not in _CACHE:
        _CACHE[window] = build_nc(window)
    return _CACHE[window]


LAST_RESULTS = None


def kernel(x, w_qkv, w_o, window_size, _trace=False):
    window = int(window_size)
    nc, nmask = _get_nc(window)
    _, keys = _mask_plan(window)
    masks = _build_masks(window, keys)

    xT = np.ascontiguousarray(x.reshape(TOK, D).T).astype(BF16NP)

    inv = 1.0 / (THETA ** (np.arange(0, HD, 2, dtype=np.float64) / HD))
    freqs = np.arange(T, dtype=np.float64)[:, None] * inv[None, :]  # [T, 64]
    cosH = np.repeat(np.cos(freqs).T, 2, axis=0).astype(BF16NP)  # [128, T]
    sign = np.where(np.arange(HD) % 2 == 0, -1.0, 1.0)[:, None]
    sinH = (np.repeat(np.sin(freqs).T, 2, axis=0) * sign).astype(BF16NP)
    ident = np.eye(128).astype(BF16NP)

    in_maps = []
    for c in range(NCORES):
        wq = w_qkv[QROWS * c:QROWS * (c + 1)]
        wk = w_qkv[H * HD + HD * c: H * HD + HD * (c + 1)]
        wv = w_qkv[H * HD + G * HD + HD * c: H * HD + G * HD + HD * (c + 1)]
        wqkvT = np.ascontiguousarray(
            np.concatenate([wq, wk, wv], axis=0).T).astype(BF16NP)
        woT = np.ascontiguousarray(
            w_o[:, QROWS * c:QROWS * (c + 1)].T).astype(BF16NP)
        in_maps.append({
            "xT": xT, "wqkvT": wqkvT, "woT": woT,
            "cosH": cosH, "sinH": sinH, "masks": masks.astype(BF16NP),
            "ident": ident,
        })

    from concourse.bass_utils import run_bass_kernel_spmd
    res = run_bass_kernel_spmd(nc, in_maps, core_ids=list(range(NCORES)),
                               trace=_trace)
    global LAST_RESULTS
    LAST_RESULTS = res
    acc = res.results[0]["out"].astype(np.float32).copy()
    for c in range(1, NCORES):
        acc += res.results[c]["out"]
    return acc.reshape(B, T, D)



# revision 9
# speedup vs baseline: 1.0757x; 1.0757x over previous
"""Trainium2 Bass kernel: fused QKV + RoPE + causal/windowed GQA attention + output proj.

Sharding: tensor-parallel by head across 8 cores. Core c owns Q-heads
4c..4c+3 and KV-group c, plus the 512 w_o columns for those heads. Each
core computes a full-shape fp16 partial of the final output; the host
sums the 8 partials. No device collectives.

Single fused pipeline in fp16 (same PE rate as bf16, 8x lower error):
after P1 finishes the 256-token group n = b*8 + g2, that group's K/V
rows are final, so window (b, g2) of attention (its queries AND all its
keys) is emitted immediately — P2/P3 instructions fill PE gaps inside
P1's stream and vice versa.

  P1: qkvT[e, tok] = w^T @ xT   (w SBUF-resident e-major; x streamed)
      v computed directly in [tok, hd] orientation (lhsT = xT chunk)
      so no PE transposes are needed for PV.
  P2: per (b, g2, head): ST[k, q] pair-tiles -> exp pairs into a window
      est buffer [128, 256, nvis]; diagonal chunks masked by DVE mult;
      denominator = DVE chunk-reduce + Pool partition_all_reduce (no
      rowsum matmuls); PV accumulates o_ps; DVE normalizes into outT.
  P3: out[tok, e] = outT^T @ w_o per window, fp16 partials DMA'd out.
"""

import math
import sys
from contextlib import ExitStack

import numpy as np

sys.path.insert(0, "/opt/trn_rl_repo")

import ml_dtypes

F16NP = np.float16

import concourse.bass as bass
import concourse.mybir as mybir
import concourse.tile as tile
from concourse import bacc
from concourse import bass_isa

F32 = mybir.dt.float32
F16 = mybir.dt.float16

B, T, D = 2, 2048, 4096
H, G, HD = 32, 8, 128
THETA = 10000.0
NCORES = 8
HL = H // NCORES            # 4 local q heads
TOK = B * T                 # 4096
QROWS = HL * HD             # 512 local q rows
E = QROWS + 2 * HD          # 768 local qkv rows
SCALE = 1.0 / math.sqrt(HD)

TOKG = 256                  # P1 token-group width == P2 query-group width
NG = TOK // TOKG            # 16 groups; group n = (b= n//8, g2= n%8)
NGB = T // TOKG             # 8 groups per batch
NDC = D // 128              # 32 contraction chunks
NE = E // 128               # 6 qkv row chunks (4 q, 1 k, 1 v)
NKC = T // 128              # 16 key chunks per batch


def _mask_plan(window: int):
    """Per (g2, kc): 'skip', 'full', or mask-key (i-j offset based).

    Chunks are always computed full-width (256 queries); masked chunks
    multiply by a {0,1} mask afterward, so the est buffer holds exact
    zeros outside the visible region.
    """
    plan = {}
    keys = {}
    for g in range(NGB):
        for kc in range(NKC):
            i_min, i_max = TOKG * g, TOKG * g + TOKG - 1
            j_min, j_max = 128 * kc, 128 * kc + 127
            if j_min > i_max or (i_min - j_max) >= window:
                plan[(g, kc)] = ("skip", None)
            elif j_max <= i_min and (i_max - j_min) < window:
                plan[(g, kc)] = ("full", None)
            else:
                key = TOKG * g - 128 * kc
                if key not in keys:
                    keys[key] = len(keys)
                plan[(g, kc)] = ("mask", keys[key])
    return plan, keys


def _build_masks(window: int, keys: dict) -> np.ndarray:
    n = max(1, len(keys))
    m = np.zeros((n, 128, TOKG), dtype=np.float32)
    for key, idx in keys.items():
        qq = np.arange(TOKG)[None, :]
        kk = np.arange(128)[:, None]
        diff = key + qq - kk          # i - j
        vis = (diff >= 0) & (diff < window)
        m[idx] = np.where(vis, 1.0, 0.0)
    return m


PAIRSWAP = [i ^ 1 for i in range(32)]


def _rope_ops(nc, pool, dst, src, cos_ap, sin_ap):
    """Interleaved-pair RoPE: dst = src*cos + pairswap(src)*signed_sin.

    cos_ap rows (2i, 2i+1) hold cos_i; sin_ap rows hold (-sin_i, +sin_i).
    src may alias dst (in-place).
    """
    W = dst.shape[-1]
    sw = pool.tile([128, W], F16, tag="rope_sw", name="rope_sw")
    tmp = pool.tile([128, W], F16, tag="rope_tmp", name="rope_tmp")
    qc = pool.tile([128, W], F16, tag="rope_qc", name="rope_qc")
    mult = mybir.AluOpType.mult
    nc.vector.stream_shuffle(sw, src, PAIRSWAP)
    nc.vector.tensor_tensor(tmp, sw, sin_ap, mult)
    nc.vector.tensor_tensor(qc, src, cos_ap, mult)
    nc.vector.tensor_tensor(dst, qc, tmp, mybir.AluOpType.add)


def build_nc(window: int):
    plan, keys = _mask_plan(window)
    nmask = max(1, len(keys))

    nc = bacc.Bacc()
    xT_d = nc.dram_tensor("xT", [D, TOK], F16, kind="ExternalInput")
    # host-packed e-major: [e_chunk, partition, dc, 128]
    w_d = nc.dram_tensor("wE", [NE, 128, NDC, 128], F16, kind="ExternalInput")
    # host-packed per local head: [hd_chunk, partition, D]
    wo_d = nc.dram_tensor("woT", [HL, 128, D], F16, kind="ExternalInput")
    cos_d = nc.dram_tensor("cosH", [128, T], F16, kind="ExternalInput")
    sin_d = nc.dram_tensor("sinH", [128, T], F16, kind="ExternalInput")
    masks_d = nc.dram_tensor("masks", [nmask, 128, TOKG], F16, kind="ExternalInput")
    out_d = nc.dram_tensor("out", [TOK, D], F16, kind="ExternalOutput")

    with ExitStack() as octx:
        tc = octx.enter_context(tile.TileContext(nc))
        # persistent SBUF
        pers = octx.enter_context(tc.tile_pool(name="pers", bufs=1))
        wsb = pers.tile([128, NE, NDC, 128], F16, name="wsb")
        qkvT = [pers.tile([128, TOK], F16, tag=f"qkv{e}", name=f"qkv{e}")
                for e in range(5)]               # 4 q heads + k
        ksb = qkvT[HL]
        vsb = pers.tile([128, TOK // 128, 128], F16, name="vsb")
        cos_sb = pers.tile([128, T], F16, name="cos_sb")
        sin_sb = pers.tile([128, T], F16, name="sin_sb")
        mask_sb = pers.tile([128, nmask, TOKG], F16, name="mask_sb")
        wo = [pers.tile([128, D], F16, tag=f"wo{dc}", name=f"wo{dc}")
              for dc in range(HL)]

        # working pools
        xp = octx.enter_context(tc.tile_pool(name="xp", bufs=2))
        ep = octx.enter_context(tc.tile_pool(name="ep", bufs=2))
        rp = octx.enter_context(tc.tile_pool(name="rp", bufs=2))
        op = octx.enter_context(tc.tile_pool(name="op", bufs=2))
        ps_p1 = octx.enter_context(tc.tile_pool(name="ps1", bufs=3, space="PSUM"))
        ps_st = octx.enter_context(tc.tile_pool(name="ps_st", bufs=1, space="PSUM"))
        ps_o = octx.enter_context(tc.tile_pool(name="ps_o", bufs=2, space="PSUM"))
        ps_p3 = octx.enter_context(tc.tile_pool(name="ps3", bufs=2, space="PSUM"))

        def window_head(n, hh, outTw):
            """Emit one head's QK->exp->PV->denominator->normalize chain."""
            b, g2 = divmod(n, NGB)
            t0 = n * TOKG
            vis = [(kc, plan[(g2, kc)]) for kc in range(NKC)
                   if plan[(g2, kc)][0] != "skip"]
            nvis = len(vis)
            if True:
                esw = ep.tile([128, TOKG, nvis], F16, tag="esw", name="esw",
                              padded_shape=[128, TOKG, NKC])
                o_ps = ps_o.tile([128, TOKG], F32, tag="o", name="o_ps")
                qtile = qkvT[hh][:, t0:t0 + TOKG]
                racc1 = None
                # QK + exp in pairs sharing one PSUM bank
                i = 0
                while i < nvis:
                    npair = min(2, nvis - i)
                    st = ps_st.tile([128, 2, TOKG], F32, tag="st", name="st")
                    for j in range(npair):
                        kc = vis[i + j][0]
                        nc.tensor.matmul(
                            st[:, j, :],
                            lhsT=ksb[:, b * T + kc * 128:b * T + (kc + 1) * 128],
                            rhs=qtile,
                            start=True, stop=True)
                    nc.scalar.activation(
                        esw[:, :, i:i + npair].rearrange("p q s -> p s q"),
                        st[:, 0:npair, :],
                        mybir.ActivationFunctionType.Exp)
                    # mask chunks in this pair (diagonal / window edges)
                    for j in range(npair):
                        kind, mid = vis[i + j][1]
                        if kind == "mask":
                            sl = esw[:, :, i + j]
                            nc.vector.tensor_tensor(
                                sl, sl, mask_sb[:, mid, :],
                                mybir.AluOpType.mult)
                    i += npair
                    # early partial denominator: everything but the last pair
                    # leaves only a short reduce on the window's tail chain
                    if i == nvis - 2 and nvis > 2:
                        racc1 = rp.tile([128, TOKG], F16, tag="racc1",
                                        name="racc1")
                        with nc.allow_low_precision(
                                "est chunk-reduce; denominator finished in f32"):
                            nc.vector.tensor_reduce(
                                racc1, esw[:, :, 0:i], mybir.AxisListType.X,
                                mybir.AluOpType.add)
                # PV accumulation (trim pure-causal zero prefix columns)
                for idx, (kc, (kind, mid)) in enumerate(vis):
                    qlo = max(0, 128 * kc - TOKG * g2)
                    nc.tensor.matmul(
                        o_ps[:, qlo:TOKG],
                        lhsT=vsb[:, b * NKC + kc, :],
                        rhs=esw[:, qlo:TOKG, idx],
                        start=(idx == 0), stop=(idx == nvis - 1))
                # denominator: chunk-reduce (DVE) + partition reduce (Pool)
                racc = rp.tile([128, TOKG], F32, tag="racc", name="racc")
                with nc.allow_low_precision(
                        "fp16 partial + f32 combine for denominator"):
                    if racc1 is not None:
                        racc2 = rp.tile([128, TOKG], F16, tag="racc2",
                                        name="racc2")
                        nc.vector.tensor_reduce(
                            racc2, esw[:, :, nvis - 2:nvis],
                            mybir.AxisListType.X, mybir.AluOpType.add)
                        nc.vector.tensor_tensor(
                            racc, racc1, racc2, mybir.AluOpType.add)
                    else:
                        nc.vector.tensor_reduce(
                            racc, esw, mybir.AxisListType.X,
                            mybir.AluOpType.add)
                rsum = rp.tile([128, TOKG], F32, tag="rsum", name="rsum")
                nc.gpsimd.partition_all_reduce(
                    rsum, racc, channels=128, reduce_op=bass_isa.ReduceOp.add)
                rrec = rp.tile([128, TOKG], F32, tag="rrec", name="rrec")
                nc.vector.reciprocal(rrec, rsum)
                ow = op.tile([128, TOKG], F16, tag=f"outw{hh}", name="ow")
                nc.vector.tensor_tensor(ow, o_ps, rrec, mybir.AluOpType.mult)
                outTw.append(ow)

        def window_p3(n, tloc, outTw):
            """Output projection for one 128-token chunk of window n."""
            t0 = n * TOKG
            tch = t0 // 128 + tloc
            for et in range(D // 512):
                ps = ps_p3.tile([128, 512], F32, tag="p3", name="p3ps")
                for dc in range(HL):
                    nc.tensor.matmul(
                        ps,
                        lhsT=outTw[dc][:, tloc * 128:(tloc + 1) * 128],
                        rhs=wo[dc][:, et * 512:(et + 1) * 512],
                        start=(dc == 0), stop=(dc == HL - 1))
                pan = op.tile([128, 512], F16, tag="pan", name="pan")
                if et % 2 == 0:
                    nc.scalar.copy(pan, ps)
                else:
                    nc.vector.tensor_scalar_add(pan, ps, 0.0)
                nc.sync.dma_start(
                    out=out_d[tch * 128:(tch + 1) * 128,
                              et * 512:(et + 1) * 512],
                    in_=pan)

        # --- prologue DMAs, just-in-time order (transfers serialize) ---
        x_r = xT_d[:].rearrange("(dc p) t -> p dc t", p=128)

        def x_slab(n, name):
            xs = xp.tile([128, NDC, TOKG], F16, tag="xslab", name=name)
            for dq in range(4):
                nc.sync.dma_start(
                    out=xs[:, dq * 8:(dq + 1) * 8, :],
                    in_=x_r[:, dq * 8:(dq + 1) * 8,
                            n * TOKG:(n + 1) * TOKG])
            return xs

        xsb0 = xp.tile([128, NDC, TOKG], F16, tag="xslab", name="xsb0")
        nc.sync.dma_start(out=xsb0[:, 0:8, :], in_=x_r[:, 0:8, 0:TOKG])
        for dw in range(4):                   # w e=0 in quarters
            nc.sync.dma_start(out=wsb[:, 0, dw * 8:(dw + 1) * 8, :],
                              in_=w_d[0, :, dw * 8:(dw + 1) * 8, :])
        for dq in range(1, 4):
            nc.sync.dma_start(out=xsb0[:, dq * 8:(dq + 1) * 8, :],
                              in_=x_r[:, dq * 8:(dq + 1) * 8, 0:TOKG])
        for e in range(1, NE):
            nc.sync.dma_start(out=wsb[:, e, :, :], in_=w_d[e])
        xpre = {0: xsb0, 1: x_slab(1, "xsb1")}
        nc.sync.dma_start(out=cos_sb, in_=cos_d[:])
        nc.sync.dma_start(out=sin_sb, in_=sin_d[:])
        nc.sync.dma_start(out=mask_sb,
                          in_=masks_d[:].rearrange("n p q -> p n q"))
        xpre[2] = x_slab(2, "xsb2")
        for dc in range(HL):
            nc.sync.dma_start(out=wo[dc], in_=wo_d[dc])
        xpre[3] = x_slab(3, "xsb3")

        def p1_phase(n, e, xsb_cur):
            """One eviction-unit of P1: e in 0..4 -> qkv row chunk;
            e == 5 -> both v token chunks."""
            t0 = n * TOKG
            if e < 5:
                ps = ps_p1.tile([128, TOKG], F32, tag="p1", name="p1ps")
                for dc in range(NDC):
                    nc.tensor.matmul(
                        ps,
                        lhsT=wsb[:, e, dc, :],
                        rhs=xsb_cur[:, dc, :],
                        start=(dc == 0), stop=(dc == NDC - 1))
                # fold softmax 1/sqrt(HD) into q rows at eviction
                nc.scalar.mul(qkvT[e][:, t0:t0 + TOKG], ps,
                              SCALE if e < HL else 1.0)
            else:
                for tch in range(TOKG // 128):  # v in [tok, hd] orientation
                    vp = ps_p1.tile([128, 128], F32, tag="p1", name="vps")
                    for dc in range(NDC):
                        nc.tensor.matmul(
                            vp,
                            lhsT=xsb_cur[:, dc, tch * 128:(tch + 1) * 128],
                            rhs=wsb[:, 5, dc, :],
                            start=(dc == 0), stop=(dc == NDC - 1))
                    nc.scalar.copy(vsb[:, t0 // 128 + tch, :], vp)

        def rope_one(n, e):
            """In-place RoPE of group n's row-chunk e (0..3 q heads, 4 k)."""
            g2 = n % NGB
            t0 = n * TOKG
            cs = cos_sb[:, g2 * TOKG:(g2 + 1) * TOKG]
            sn = sin_sb[:, g2 * TOKG:(g2 + 1) * TOKG]
            _rope_ops(nc, op, qkvT[e][:, t0:t0 + TOKG],
                      qkvT[e][:, t0:t0 + TOKG], cs, sn)

        xsb_cur = xsb0
        prevw = None        # (window_n, outTw) with pending pieces
        for n in range(NG):
            # prefetch next x slab (first few were issued in the prologue)
            if n + 1 < NG and n + 1 not in xpre:
                xpre[n + 1] = x_slab(n + 1, f"xsb{n + 1}")
            xsb_nxt = xpre.get(n + 1)

            last = n == NG - 1
            # last group: k and q first with inline rope, so the final
            # window's exp chains start as early as possible
            e_order = (4, 0, 1, 2, 3, 5) if last else (0, 1, 2, 3, 4, 5)
            outTw = []
            for step, e in enumerate(e_order):
                p1_phase(n, e, xsb_cur)
                if last and e != 5:
                    rope_one(n, e)
                # window n-1 pieces between P1 eviction units
                if prevw is not None and step < 6:
                    wn, wout = prevw
                    if step < 4:
                        window_head(wn, step, wout)
                    elif step == 4:
                        window_p3(wn, 0, wout)
                    else:
                        window_p3(wn, 1, wout)
            prevw = None
            xsb_cur = xsb_nxt if n + 1 < NG else None

            if not last:
                # RoPE in place for this group (k first: QK needs it)
                rope_one(n, 4)
                for hh in range(HL):
                    rope_one(n, hh)
            prevw = (n, outTw)

        # final window: heads + P3, nothing left to interleave
        wn, wout = prevw
        for hh in range(HL):
            window_head(wn, hh, wout)
        window_p3(wn, 0, wout)
        window_p3(wn, 1, wout)

    nc.finalize()
    return nc, nmask


_CACHE = {}


def _get_nc(window: int):
    if window not in _CACHE:
        _CACHE[window] = build_nc(window)
    return _CACHE[window]


LAST_RESULTS = None


def kernel(x, w_qkv, w_o, window_size, _trace=False):
    window = int(window_size)
    nc, nmask = _get_nc(window)
    _, keys = _mask_plan(window)
    masks = _build_masks(window, keys)

    xT = np.ascontiguousarray(x.reshape(TOK, D).T).astype(F16NP)

    inv = 1.0 / (THETA ** (np.arange(0, HD, 2, dtype=np.float64) / HD))
    freqs = np.arange(T, dtype=np.float64)[:, None] * inv[None, :]  # [T, 64]
    cosH = np.repeat(np.cos(freqs).T, 2, axis=0).astype(F16NP)  # [128, T]
    sign = np.where(np.arange(HD) % 2 == 0, -1.0, 1.0)[:, None]
    sinH = (np.repeat(np.sin(freqs).T, 2, axis=0) * sign).astype(F16NP)

    in_maps = []
    for c in range(NCORES):
        wq = w_qkv[QROWS * c:QROWS * (c + 1)]
        wk = w_qkv[H * HD + HD * c: H * HD + HD * (c + 1)]
        wv = w_qkv[H * HD + G * HD + HD * c: H * HD + G * HD + HD * (c + 1)]
        wqkvT = np.concatenate([wq, wk, wv], axis=0).T  # [D, E] f32
        # e-major pack: [e_chunk, partition(=dc inner 128 rows), dc, 128]
        wE = np.ascontiguousarray(
            wqkvT.reshape(NDC, 128, NE, 128).transpose(2, 1, 0, 3)
        ).astype(F16NP)
        woT = np.ascontiguousarray(
            w_o[:, QROWS * c:QROWS * (c + 1)].T.reshape(HL, 128, D)
        ).astype(F16NP)
        in_maps.append({
            "xT": xT, "wE": wE, "woT": woT,
            "cosH": cosH, "sinH": sinH, "masks": masks.astype(F16NP),
        })

    from concourse.bass_utils import run_bass_kernel_spmd
    res = run_bass_kernel_spmd(nc, in_maps, core_ids=list(range(NCORES)),
                               trace=_trace)
    global LAST_RESULTS
    LAST_RESULTS = res
    acc = res.results[0]["out"].astype(np.float32)
    for c in range(1, NCORES):
        acc = acc + res.results[c]["out"].astype(np.float32)
    return acc.reshape(B, T, D)


# revision 19
# speedup vs baseline: 1.0894x; 1.0127x over previous
"""Trainium2 Bass kernel: fused QKV + RoPE + causal/windowed GQA attention + output proj.

Sharding: tensor-parallel by head across 8 cores. Core c owns Q-heads
4c..4c+3 and KV-group c, plus the 512 w_o columns for those heads. Each
core computes a full-shape fp16 partial of the final output; the host
sums the 8 partials. No device collectives.

Single fused pipeline in fp16 (same PE rate as bf16, 8x lower error):
after P1 finishes the 256-token group n = b*8 + g2, that group's K/V
rows are final, so window (b, g2) of attention (its queries AND all its
keys) is emitted immediately — P2/P3 instructions fill PE gaps inside
P1's stream and vice versa.

  P1: qkvT[e, tok] = w^T @ xT   (w SBUF-resident e-major; x streamed)
      v computed directly in [tok, hd] orientation (lhsT = xT chunk)
      so no PE transposes are needed for PV.
  P2: per (b, g2, head): ST[k, q] pair-tiles -> exp pairs into a window
      est buffer [128, 256, nvis]; diagonal chunks masked by DVE mult;
      denominator = DVE chunk-reduce + Pool partition_all_reduce (no
      rowsum matmuls); PV accumulates o_ps; DVE normalizes into outT.
  P3: out[tok, e] = outT^T @ w_o per window, fp16 partials DMA'd out.
"""

import math
import sys
from contextlib import ExitStack

import numpy as np

sys.path.insert(0, "/opt/trn_rl_repo")

import ml_dtypes

F16NP = np.float16

import concourse.bass as bass
import concourse.mybir as mybir
import concourse.tile as tile
from concourse import bacc
from concourse import bass_isa

F32 = mybir.dt.float32
F16 = mybir.dt.float16

B, T, D = 2, 2048, 4096
H, G, HD = 32, 8, 128
THETA = 10000.0
NCORES = 8
HL = H // NCORES            # 4 local q heads
TOK = B * T                 # 4096
QROWS = HL * HD             # 512 local q rows
E = QROWS + 2 * HD          # 768 local qkv rows
SCALE = 1.0 / math.sqrt(HD)

TOKG = 256                  # P1 token-group width == P2 query-group width
NG = TOK // TOKG            # 16 groups; group n = (b= n//8, g2= n%8)
NGB = T // TOKG             # 8 groups per batch
NDC = D // 128              # 32 contraction chunks
NE = E // 128               # 6 qkv row chunks (4 q, 1 k, 1 v)
NKC = T // 128              # 16 key chunks per batch


def _mask_plan(window: int):
    """Per (g2, kc): 'skip', 'full', or mask-key (i-j offset based).

    Chunks are always computed full-width (256 queries); masked chunks
    multiply by a {0,1} mask afterward, so the est buffer holds exact
    zeros outside the visible region.
    """
    plan = {}
    keys = {}
    for g in range(NGB):
        for kc in range(NKC):
            i_min, i_max = TOKG * g, TOKG * g + TOKG - 1
            j_min, j_max = 128 * kc, 128 * kc + 127
            if j_min > i_max or (i_min - j_max) >= window:
                plan[(g, kc)] = ("skip", None)
            elif j_max <= i_min and (i_max - j_min) < window:
                plan[(g, kc)] = ("full", None)
            else:
                key = TOKG * g - 128 * kc
                if key not in keys:
                    keys[key] = len(keys)
                plan[(g, kc)] = ("mask", keys[key])
    return plan, keys


def _build_masks(window: int, keys: dict) -> np.ndarray:
    n = max(1, len(keys))
    m = np.zeros((n, 128, TOKG), dtype=np.float32)
    for key, idx in keys.items():
        qq = np.arange(TOKG)[None, :]
        kk = np.arange(128)[:, None]
        diff = key + qq - kk          # i - j
        vis = (diff >= 0) & (diff < window)
        m[idx] = np.where(vis, 1.0, 0.0)
    return m


PAIRSWAP = [i ^ 1 for i in range(32)]


def _rope_ops(nc, pool, dst, src, cos_ap, sin_ap):
    """Interleaved-pair RoPE: dst = src*cos + pairswap(src)*signed_sin.

    cos_ap rows (2i, 2i+1) hold cos_i; sin_ap rows hold (-sin_i, +sin_i).
    src may alias dst (in-place).
    """
    W = dst.shape[-1]
    sw = pool.tile([128, W], F16, tag="rope_sw", name="rope_sw")
    tmp = pool.tile([128, W], F16, tag="rope_tmp", name="rope_tmp")
    qc = pool.tile([128, W], F16, tag="rope_qc", name="rope_qc")
    mult = mybir.AluOpType.mult
    nc.vector.stream_shuffle(sw, src, PAIRSWAP)
    nc.vector.tensor_tensor(tmp, sw, sin_ap, mult)
    nc.vector.tensor_tensor(qc, src, cos_ap, mult)
    nc.vector.tensor_tensor(dst, qc, tmp, mybir.AluOpType.add)


def build_nc(window: int):
    plan, keys = _mask_plan(window)
    nmask = max(1, len(keys))

    nc = bacc.Bacc()
    xT_d = nc.dram_tensor("xT", [D, TOK], F16, kind="ExternalInput")
    # host-packed e-major: [e_chunk, partition, dc, 128]
    w_d = nc.dram_tensor("wE", [NE, 128, NDC, 128], F16, kind="ExternalInput")
    # host-packed per local head: [hd_chunk, partition, D]
    wo_d = nc.dram_tensor("woT", [HL, 128, D], F16, kind="ExternalInput")
    cos_d = nc.dram_tensor("cosH", [128, T], F16, kind="ExternalInput")
    sin_d = nc.dram_tensor("sinH", [128, T], F16, kind="ExternalInput")
    masks_d = nc.dram_tensor("masks", [nmask, 128, TOKG], F16, kind="ExternalInput")
    out_d = nc.dram_tensor("out", [TOK, D], F16, kind="ExternalOutput")

    with ExitStack() as octx:
        tc = octx.enter_context(tile.TileContext(nc))
        # persistent SBUF
        pers = octx.enter_context(tc.tile_pool(name="pers", bufs=1))
        wsb = pers.tile([128, NE, NDC, 128], F16, name="wsb")
        qkvT = [pers.tile([128, TOK], F16, tag=f"qkv{e}", name=f"qkv{e}")
                for e in range(5)]               # 4 q heads + k
        ksb = qkvT[HL]
        vsb = pers.tile([128, TOK // 128, 128], F16, name="vsb")
        cos_sb = pers.tile([128, T], F16, name="cos_sb")
        sin_sb = pers.tile([128, T], F16, name="sin_sb")
        mask_sb = pers.tile([128, nmask, TOKG], F16, name="mask_sb")
        wo = [pers.tile([128, D], F16, tag=f"wo{dc}", name=f"wo{dc}")
              for dc in range(HL)]
        ones_sb = pers.tile([128, 1], F16, name="ones_sb")
        nc.vector.memset(ones_sb, 1.0)

        # working pools
        xp = octx.enter_context(tc.tile_pool(name="xp", bufs=2))
        ep = octx.enter_context(tc.tile_pool(name="ep", bufs=2))
        rp = octx.enter_context(tc.tile_pool(name="rp", bufs=2))
        op = octx.enter_context(tc.tile_pool(name="op", bufs=2))
        ps_p1 = octx.enter_context(tc.tile_pool(name="ps1", bufs=3, space="PSUM"))
        ps_st = octx.enter_context(tc.tile_pool(name="ps_st", bufs=1, space="PSUM"))
        ps_o = octx.enter_context(tc.tile_pool(name="ps_o", bufs=2, space="PSUM"))
        ps_p3 = octx.enter_context(tc.tile_pool(name="ps3", bufs=2, space="PSUM"))

        def window_head(n, hh, outTw, tail=False):
            """Emit one head's QK->exp->PV->denominator->normalize chain.

            tail=True (final window, P1 finished): steal idle ps_p1 banks
            as extra score buffers so QK pairs don't serialize on exp, and
            compute the denominator with PE rowsum matmuls (PE has idle
            slack there, DVE is the tail critical path)."""
            b, g2 = divmod(n, NGB)
            t0 = n * TOKG
            vis = [(kc, plan[(g2, kc)]) for kc in range(NKC)
                   if plan[(g2, kc)][0] != "skip"]
            nvis = len(vis)
            if True:
                esw = ep.tile([128, TOKG, nvis], F16, tag="esw", name="esw",
                              padded_shape=[128, TOKG, NKC])
                o_ps = ps_o.tile([128, TOKG], F32, tag="o", name="o_ps")
                qtile = qkvT[hh][:, t0:t0 + TOKG]
                racc1 = None
                # QK + exp in pairs sharing one PSUM bank
                i = 0
                while i < nvis:
                    npair = min(2, nvis - i)
                    stp = ps_p1 if tail else ps_st
                    st = stp.tile([128, 2, TOKG], F32,
                                  tag="p1" if stp is ps_p1 else "st",
                                  name="st")
                    for j in range(npair):
                        kc = vis[i + j][0]
                        nc.tensor.matmul(
                            st[:, j, :],
                            lhsT=ksb[:, b * T + kc * 128:b * T + (kc + 1) * 128],
                            rhs=qtile,
                            start=True, stop=True)
                    nc.scalar.activation(
                        esw[:, :, i:i + npair].rearrange("p q s -> p s q"),
                        st[:, 0:npair, :],
                        mybir.ActivationFunctionType.Exp)
                    # mask chunks in this pair (diagonal / window edges)
                    for j in range(npair):
                        kind, mid = vis[i + j][1]
                        if kind == "mask":
                            sl = esw[:, :, i + j]
                            nc.vector.tensor_tensor(
                                sl, sl, mask_sb[:, mid, :],
                                mybir.AluOpType.mult)
                    i += npair
                    # early partial denominator: everything but the last pair
                    # leaves only a short reduce on the window's tail chain
                    if i == nvis - 2 and nvis > 2:
                        racc1 = rp.tile([128, TOKG], F16, tag="racc1",
                                        name="racc1")
                        with nc.allow_low_precision(
                                "est chunk-reduce; denominator finished in f32"):
                            nc.vector.tensor_reduce(
                                racc1, esw[:, :, 0:i], mybir.AxisListType.X,
                                mybir.AluOpType.add)
                if tail and False:
                    # PE rowsum denominator + broadcast (keeps DVE short)
                    r_ps = ps_p3.tile([1, TOKG], F32, tag="p3", name="r_ps")
                    for idx in range(nvis):
                        nc.tensor.matmul(
                            r_ps, lhsT=ones_sb, rhs=esw[:, :, idx],
                            start=(idx == 0), stop=(idx == nvis - 1))
                    rrec1 = rp.tile([1, TOKG], F32, tag="rrec1", name="rrec1")
                    nc.vector.reciprocal(rrec1, r_ps)
                    rb = rp.tile([128, TOKG], F32, tag="rb", name="rb")
                    nc.gpsimd.partition_broadcast(rb, rrec1)
                    ow = op.tile([128, TOKG], F16, tag=f"outw{hh}", name="ow")
                    nc.vector.tensor_tensor(ow, o_ps, rb,
                                            mybir.AluOpType.mult)
                    outTw.append(ow)
                    return
                # finish the denominator (short tail after the last exp)
                acc = rp.tile([128, TOKG], F32, tag="acc", name="acc")
                with nc.allow_low_precision(
                        "fp16 partial + f32 combine for denominator"):
                    if racc1 is not None:
                        racc2 = rp.tile([128, TOKG], F16, tag="racc2",
                                        name="racc2")
                        nc.vector.tensor_reduce(
                            racc2, esw[:, :, nvis - 2:nvis],
                            mybir.AxisListType.X, mybir.AluOpType.add)
                        nc.vector.tensor_tensor(
                            acc, racc1, racc2, mybir.AluOpType.add)
                    else:
                        nc.vector.tensor_reduce(
                            acc, esw, mybir.AxisListType.X,
                            mybir.AluOpType.add)
                # PV accumulation (trim pure-causal zero prefix columns)
                for idx, (kc, (kind, mid)) in enumerate(vis):
                    qlo = max(0, 128 * kc - TOKG * g2)
                    nc.tensor.matmul(
                        o_ps[:, qlo:TOKG],
                        lhsT=vsb[:, b * NKC + kc, :],
                        rhs=esw[:, qlo:TOKG, idx],
                        start=(idx == 0), stop=(idx == nvis - 1))
                # partition reduce of the accumulated denominator (Pool)
                rsum = rp.tile([128, TOKG], F32, tag="rsum", name="rsum")
                nc.gpsimd.partition_all_reduce(
                    rsum, acc, channels=128, reduce_op=bass_isa.ReduceOp.add)
                rrec = rp.tile([128, TOKG], F32, tag="rrec", name="rrec")
                nc.vector.reciprocal(rrec, rsum)
                ow = op.tile([128, TOKG], F16, tag=f"outw{hh}", name="ow")
                nc.vector.tensor_tensor(ow, o_ps, rrec, mybir.AluOpType.mult)
                outTw.append(ow)

        def window_p3(n, tloc, outTw):
            """Output projection for one 128-token chunk of window n."""
            t0 = n * TOKG
            tch = t0 // 128 + tloc
            for et in range(D // 512):
                ps = ps_p3.tile([128, 512], F32, tag="p3", name="p3ps")
                for dc in range(HL):
                    nc.tensor.matmul(
                        ps,
                        lhsT=outTw[dc][:, tloc * 128:(tloc + 1) * 128],
                        rhs=wo[dc][:, et * 512:(et + 1) * 512],
                        start=(dc == 0), stop=(dc == HL - 1))
                pan = op.tile([128, 512], F16, tag="pan", name="pan")
                if et % 2 == 0:
                    nc.scalar.copy(pan, ps)
                else:
                    nc.vector.tensor_scalar_add(pan, ps, 0.0)
                nc.sync.dma_start(
                    out=out_d[tch * 128:(tch + 1) * 128,
                              et * 512:(et + 1) * 512],
                    in_=pan)

        # --- prologue DMAs, just-in-time order (transfers serialize) ---
        x_r = xT_d[:].rearrange("(dc p) t -> p dc t", p=128)

        def x_slab(n, name):
            xs = xp.tile([128, NDC, TOKG], F16, tag="xslab", name=name)
            for dq in range(4):
                nc.sync.dma_start(
                    out=xs[:, dq * 8:(dq + 1) * 8, :],
                    in_=x_r[:, dq * 8:(dq + 1) * 8,
                            n * TOKG:(n + 1) * TOKG])
            return xs

        xsb0 = xp.tile([128, NDC, TOKG], F16, tag="xslab", name="xsb0")
        nc.sync.dma_start(out=xsb0[:, 0:8, :], in_=x_r[:, 0:8, 0:TOKG])
        for dw in range(4):                   # w e=0 in quarters
            nc.sync.dma_start(out=wsb[:, 0, dw * 8:(dw + 1) * 8, :],
                              in_=w_d[0, :, dw * 8:(dw + 1) * 8, :])
        for dq in range(1, 4):
            nc.sync.dma_start(out=xsb0[:, dq * 8:(dq + 1) * 8, :],
                              in_=x_r[:, dq * 8:(dq + 1) * 8, 0:TOKG])
        for e in range(1, NE):
            nc.sync.dma_start(out=wsb[:, e, :, :], in_=w_d[e])
        xpre = {0: xsb0, 1: x_slab(1, "xsb1")}
        nc.sync.dma_start(out=cos_sb, in_=cos_d[:])
        nc.sync.dma_start(out=sin_sb, in_=sin_d[:])
        nc.sync.dma_start(out=mask_sb,
                          in_=masks_d[:].rearrange("n p q -> p n q"))
        xpre[2] = x_slab(2, "xsb2")
        for dc in range(HL):
            nc.sync.dma_start(out=wo[dc], in_=wo_d[dc])
        xpre[3] = x_slab(3, "xsb3")

        def p1_phase(n, e, xsb_cur):
            """One eviction-unit of P1: e in 0..4 -> qkv row chunk;
            e == 5 -> both v token chunks."""
            t0 = n * TOKG
            if e < 5:
                ps = ps_p1.tile([128, TOKG], F32, tag="p1", name="p1ps")
                for dc in range(NDC):
                    nc.tensor.matmul(
                        ps,
                        lhsT=wsb[:, e, dc, :],
                        rhs=xsb_cur[:, dc, :],
                        start=(dc == 0), stop=(dc == NDC - 1))
                # fold softmax 1/sqrt(HD) into q rows at eviction
                nc.scalar.mul(qkvT[e][:, t0:t0 + TOKG], ps,
                              SCALE if e < HL else 1.0)
            else:
                for tch in range(TOKG // 128):  # v in [tok, hd] orientation
                    vp = ps_p1.tile([128, 128], F32, tag="p1", name="vps")
                    for dc in range(NDC):
                        nc.tensor.matmul(
                            vp,
                            lhsT=xsb_cur[:, dc, tch * 128:(tch + 1) * 128],
                            rhs=wsb[:, 5, dc, :],
                            start=(dc == 0), stop=(dc == NDC - 1))
                    nc.scalar.copy(vsb[:, t0 // 128 + tch, :], vp)

        def rope_one(n, e):
            """In-place RoPE of group n's row-chunk e (0..3 q heads, 4 k)."""
            g2 = n % NGB
            t0 = n * TOKG
            cs = cos_sb[:, g2 * TOKG:(g2 + 1) * TOKG]
            sn = sin_sb[:, g2 * TOKG:(g2 + 1) * TOKG]
            _rope_ops(nc, op, qkvT[e][:, t0:t0 + TOKG],
                      qkvT[e][:, t0:t0 + TOKG], cs, sn)

        xsb_cur = xsb0
        prevw = None        # (window_n, outTw) with pending pieces
        for n in range(NG):
            # prefetch next x slab (first few were issued in the prologue)
            if n + 1 < NG and n + 1 not in xpre:
                xpre[n + 1] = x_slab(n + 1, f"xsb{n + 1}")
            xsb_nxt = xpre.get(n + 1)

            last = n == NG - 1
            # last group: k and q first with inline rope, so the final
            # window's exp chains start as early as possible
            e_order = (4, 0, 1, 2, 3, 5) if last else (0, 1, 2, 3, 4, 5)
            outTw = []
            for step, e in enumerate(e_order):
                p1_phase(n, e, xsb_cur)
                if last and e != 5:
                    rope_one(n, e)
                # window n-1 pieces between P1 eviction units
                if prevw is not None and step < 6:
                    wn, wout = prevw
                    if step < 4:
                        window_head(wn, step, wout)
                    elif step == 4:
                        window_p3(wn, 0, wout)
                    else:
                        window_p3(wn, 1, wout)
            prevw = None
            xsb_cur = xsb_nxt if n + 1 < NG else None

            if not last:
                # RoPE in place for this group (k first: QK needs it)
                rope_one(n, 4)
                for hh in range(HL):
                    rope_one(n, hh)
            prevw = (n, outTw)

        # final window: heads + P3, nothing left to interleave
        wn, wout = prevw
        for hh in range(HL):
            window_head(wn, hh, wout, tail=True)
        window_p3(wn, 0, wout)
        window_p3(wn, 1, wout)

    nc.finalize()
    return nc, nmask


_CACHE = {}


def _get_nc(window: int):
    if window not in _CACHE:
        _CACHE[window] = build_nc(window)
    return _CACHE[window]


LAST_RESULTS = None


def kernel(x, w_qkv, w_o, window_size, _trace=False):
    window = int(window_size)
    nc, nmask = _get_nc(window)
    _, keys = _mask_plan(window)
    masks = _build_masks(window, keys)

    xT = np.ascontiguousarray(x.reshape(TOK, D).T).astype(F16NP)

    inv = 1.0 / (THETA ** (np.arange(0, HD, 2, dtype=np.float64) / HD))
    freqs = np.arange(T, dtype=np.float64)[:, None] * inv[None, :]  # [T, 64]
    cosH = np.repeat(np.cos(freqs).T, 2, axis=0).astype(F16NP)  # [128, T]
    sign = np.where(np.arange(HD) % 2 == 0, -1.0, 1.0)[:, None]
    sinH = (np.repeat(np.sin(freqs).T, 2, axis=0) * sign).astype(F16NP)

    in_maps = []
    for c in range(NCORES):
        wq = w_qkv[QROWS * c:QROWS * (c + 1)]
        wk = w_qkv[H * HD + HD * c: H * HD + HD * (c + 1)]
        wv = w_qkv[H * HD + G * HD + HD * c: H * HD + G * HD + HD * (c + 1)]
        wqkvT = np.concatenate([wq, wk, wv], axis=0).T  # [D, E] f32
        # e-major pack: [e_chunk, partition(=dc inner 128 rows), dc, 128]
        wE = np.ascontiguousarray(
            wqkvT.reshape(NDC, 128, NE, 128).transpose(2, 1, 0, 3)
        ).astype(F16NP)
        woT = np.ascontiguousarray(
            w_o[:, QROWS * c:QROWS * (c + 1)].T.reshape(HL, 128, D)
        ).astype(F16NP)
        in_maps.append({
            "xT": xT, "wE": wE, "woT": woT,
            "cosH": cosH, "sinH": sinH, "masks": masks.astype(F16NP),
        })

    from concourse.bass_utils import run_bass_kernel_spmd
    res = run_bass_kernel_spmd(nc, in_maps, core_ids=list(range(NCORES)),
                               trace=_trace)
    global LAST_RESULTS
    LAST_RESULTS = res
    acc = res.results[0]["out"].astype(np.float32)
    for c in range(1, NCORES):
        acc = acc + res.results[c]["out"].astype(np.float32)
    return acc.reshape(B, T, D)


# revision 28
# speedup vs baseline: 1.1525x; 1.0579x over previous
"""Trainium2 Bass kernel: fused QKV + RoPE + causal/windowed GQA attention + output proj.

Sharding: tensor-parallel by head across 8 cores. Core c owns Q-heads
4c..4c+3 and KV-group c, plus the 512 w_o columns for those heads. Each
core computes a full-shape fp16 partial of the final output; the host
sums the 8 partials. No device collectives.

Single fused pipeline in fp16 (same PE rate as bf16, 8x lower error):
after P1 finishes the 256-token group n = b*8 + g2, that group's K/V
rows are final, so window (b, g2) of attention (its queries AND all its
keys) is emitted immediately — P2/P3 instructions fill PE gaps inside
P1's stream and vice versa.

  P1: qkvT[e, tok] = w^T @ xT   (w SBUF-resident e-major; x streamed)
      v computed directly in [tok, hd] orientation (lhsT = xT chunk)
      so no PE transposes are needed for PV.
  P2: per (b, g2, head): ST[k, q] pair-tiles -> exp pairs into a window
      est buffer [128, 256, nvis]; diagonal chunks masked by DVE mult;
      denominator = DVE chunk-reduce + Pool partition_all_reduce (no
      rowsum matmuls); PV accumulates o_ps; DVE normalizes into outT.
  P3: out[tok, e] = outT^T @ w_o per window, fp16 partials DMA'd out.
"""

import math
import sys
from contextlib import ExitStack

import numpy as np

sys.path.insert(0, "/opt/trn_rl_repo")

import ml_dtypes

F16NP = np.float16

import concourse.bass as bass
import concourse.mybir as mybir
import concourse.tile as tile
from concourse import bacc
from concourse import bass_isa

F32 = mybir.dt.float32
F16 = mybir.dt.float16

B, T, D = 2, 2048, 4096
H, G, HD = 32, 8, 128
THETA = 10000.0
NCORES = 8
HL = H // NCORES            # 4 local q heads
TOK = B * T                 # 4096
QROWS = HL * HD             # 512 local q rows
E = QROWS + 2 * HD          # 768 local qkv rows
SCALE = 1.0 / math.sqrt(HD)

TOKG = 256                  # P1 token-group width == P2 query-group width
NG = TOK // TOKG            # 16 groups; group n = (b= n//8, g2= n%8)
NGB = T // TOKG             # 8 groups per batch
NDC = D // 128              # 32 contraction chunks
NE = E // 128               # 6 qkv row chunks (4 q, 1 k, 1 v)
NKC = T // 128              # 16 key chunks per batch


def _mask_plan(window: int):
    """Per (g2, kc): 'skip', 'full', or mask-key (i-j offset based).

    Chunks are always computed full-width (256 queries); masked chunks
    multiply by a {0,1} mask afterward, so the est buffer holds exact
    zeros outside the visible region.
    """
    plan = {}
    keys = {}
    for g in range(NGB):
        for kc in range(NKC):
            i_min, i_max = TOKG * g, TOKG * g + TOKG - 1
            j_min, j_max = 128 * kc, 128 * kc + 127
            if j_min > i_max or (i_min - j_max) >= window:
                plan[(g, kc)] = ("skip", None)
            elif j_max <= i_min and (i_max - j_min) < window:
                plan[(g, kc)] = ("full", None)
            else:
                key = TOKG * g - 128 * kc
                if key not in keys:
                    keys[key] = len(keys)
                plan[(g, kc)] = ("mask", keys[key])
    return plan, keys


def _build_masks(window: int, keys: dict) -> np.ndarray:
    n = max(1, len(keys))
    m = np.zeros((n, 128, TOKG), dtype=np.float32)
    for key, idx in keys.items():
        qq = np.arange(TOKG)[None, :]
        kk = np.arange(128)[:, None]
        diff = key + qq - kk          # i - j
        vis = (diff >= 0) & (diff < window)
        m[idx] = np.where(vis, 1.0, 0.0)
    return m


PAIRSWAP = [i ^ 1 for i in range(32)]


def _rope_ops(nc, pool, dst, src, cos_ap, sin_ap):
    """Interleaved-pair RoPE: dst = src*cos + pairswap(src)*signed_sin.

    cos_ap rows (2i, 2i+1) hold cos_i; sin_ap rows hold (-sin_i, +sin_i).
    src may alias dst (in-place).
    """
    W = dst.shape[-1]
    sw = pool.tile([128, W], F16, tag="rope_sw", name="rope_sw")
    tmp = pool.tile([128, W], F16, tag="rope_tmp", name="rope_tmp")
    qc = pool.tile([128, W], F16, tag="rope_qc", name="rope_qc")
    mult = mybir.AluOpType.mult
    nc.vector.stream_shuffle(sw, src, PAIRSWAP)
    nc.vector.tensor_tensor(tmp, sw, sin_ap, mult)
    nc.vector.tensor_tensor(qc, src, cos_ap, mult)
    nc.vector.tensor_tensor(dst, qc, tmp, mybir.AluOpType.add)


def build_nc(window: int):
    plan, keys = _mask_plan(window)
    nmask = max(1, len(keys))

    nc = bacc.Bacc()
    xT_d = nc.dram_tensor("xT", [D, TOK], F16, kind="ExternalInput")
    # host-packed e-major: [e_chunk, partition, dc, 128]
    w_d = nc.dram_tensor("wE", [NE, 128, NDC, 128], F16, kind="ExternalInput")
    # host-packed per local head: [hd_chunk, partition, D]
    wo_d = nc.dram_tensor("woT", [HL, 128, D], F16, kind="ExternalInput")
    cos_d = nc.dram_tensor("cosH", [128, T], F16, kind="ExternalInput")
    sin_d = nc.dram_tensor("sinH", [128, T], F16, kind="ExternalInput")
    masks_d = nc.dram_tensor("masks", [nmask, 128, TOKG], F16, kind="ExternalInput")
    out_d = nc.dram_tensor("out", [TOK, D], F16, kind="ExternalOutput")

    with ExitStack() as octx:
        tc = octx.enter_context(tile.TileContext(nc))
        # persistent SBUF
        pers = octx.enter_context(tc.tile_pool(name="pers", bufs=1))
        wsb = pers.tile([128, NE, NDC, 128], F16, name="wsb")
        qkvT = [pers.tile([128, TOK], F16, tag=f"qkv{e}", name=f"qkv{e}")
                for e in range(5)]               # 4 q heads + k
        ksb = qkvT[HL]
        vsb = pers.tile([128, TOK // 128, 128], F16, name="vsb")
        cos_sb = pers.tile([128, T], F16, name="cos_sb")
        sin_sb = pers.tile([128, T], F16, name="sin_sb")
        mask_sb = pers.tile([128, nmask, TOKG], F16, name="mask_sb")
        wo = [pers.tile([128, D], F16, tag=f"wo{dc}", name=f"wo{dc}")
              for dc in range(HL)]
        ones_sb = pers.tile([128, 1], F16, name="ones_sb")
        nc.vector.memset(ones_sb, 1.0)

        # working pools
        xp = octx.enter_context(tc.tile_pool(name="xp", bufs=2))
        ep = octx.enter_context(tc.tile_pool(name="ep", bufs=2))
        rp = octx.enter_context(tc.tile_pool(name="rp", bufs=2))
        op = octx.enter_context(tc.tile_pool(name="op", bufs=2))
        ps_p1 = octx.enter_context(tc.tile_pool(name="ps1", bufs=2, space="PSUM"))
        ps_st = octx.enter_context(tc.tile_pool(name="ps_st", bufs=2, space="PSUM"))
        ps_o = octx.enter_context(tc.tile_pool(name="ps_o", bufs=2, space="PSUM"))
        ps_p3 = octx.enter_context(tc.tile_pool(name="ps3", bufs=2, space="PSUM"))

        def window_head(n, hh, outTw, tail=False):
            """Emit one head's QK->exp->PV->denominator->normalize chain.

            tail=True (final window, P1 finished): steal idle ps_p1 banks
            as extra score buffers so QK pairs don't serialize on exp, and
            compute the denominator with PE rowsum matmuls (PE has idle
            slack there, DVE is the tail critical path)."""
            b, g2 = divmod(n, NGB)
            t0 = n * TOKG
            vis = [(kc, plan[(g2, kc)]) for kc in range(NKC)
                   if plan[(g2, kc)][0] != "skip"]
            nvis = len(vis)
            if True:
                esw = ep.tile([128, TOKG, nvis], F16, tag="esw", name="esw",
                              padded_shape=[128, TOKG, NKC])
                o_ps = ps_o.tile([128, TOKG], F32, tag="o", name="o_ps")
                qtile = qkvT[hh][:, t0:t0 + TOKG]
                racc1 = None
                # QK + exp in pairs sharing one PSUM bank
                i = 0
                while i < nvis:
                    npair = min(2, nvis - i)
                    stp = ps_p1 if (tail and (i // 2) % 2 == 1) else ps_st
                    st = stp.tile([128, 2, TOKG], F32,
                                  tag="p1" if stp is ps_p1 else "st",
                                  name="st")
                    for j in range(npair):
                        kc = vis[i + j][0]
                        nc.tensor.matmul(
                            st[:, j, :],
                            lhsT=ksb[:, b * T + kc * 128:b * T + (kc + 1) * 128],
                            rhs=qtile,
                            start=True, stop=True)
                    nc.scalar.activation(
                        esw[:, :, i:i + npair].rearrange("p q s -> p s q"),
                        st[:, 0:npair, :],
                        mybir.ActivationFunctionType.Exp)
                    # mask chunks in this pair (diagonal / window edges)
                    for j in range(npair):
                        kind, mid = vis[i + j][1]
                        if kind == "mask":
                            sl = esw[:, :, i + j]
                            nc.vector.tensor_tensor(
                                sl, sl, mask_sb[:, mid, :],
                                mybir.AluOpType.mult)
                    i += npair
                    # early partial denominator: everything but the last pair
                    # leaves only a short reduce on the window's tail chain
                    mids_at = ([nvis // 4 * 2, nvis - 2] if tail and nvis > 6
                               else [nvis - 2])
                    if i in mids_at and nvis > 2:
                        part = rp.tile([128, TOKG], F16, tag="racc1",
                                       name="racc1")
                        lo = 0 if racc1 is None else racc1[1]
                        with nc.allow_low_precision(
                                "est chunk-reduce; denominator finished in f32"):
                            nc.vector.tensor_reduce(
                                part, esw[:, :, lo:i], mybir.AxisListType.X,
                                mybir.AluOpType.add)
                        if racc1 is None:
                            racc1 = (part, i)
                        else:
                            comb = rp.tile([128, TOKG], F16, tag="rcomb",
                                           name="racc1c")
                            nc.vector.tensor_tensor(
                                comb, racc1[0], part, mybir.AluOpType.add)
                            racc1 = (comb, i)
                if tail and False:
                    # PE rowsum denominator + broadcast (keeps DVE short)
                    r_ps = ps_p3.tile([1, TOKG], F32, tag="p3", name="r_ps")
                    for idx in range(nvis):
                        nc.tensor.matmul(
                            r_ps, lhsT=ones_sb, rhs=esw[:, :, idx],
                            start=(idx == 0), stop=(idx == nvis - 1))
                    rrec1 = rp.tile([1, TOKG], F32, tag="rrec1", name="rrec1")
                    nc.vector.reciprocal(rrec1, r_ps)
                    rb = rp.tile([128, TOKG], F32, tag="rb", name="rb")
                    nc.gpsimd.partition_broadcast(rb, rrec1)
                    ow = op.tile([128, TOKG], F16, tag=f"outw{hh}", name="ow")
                    nc.vector.tensor_tensor(ow, o_ps, rb,
                                            mybir.AluOpType.mult)
                    outTw.append(ow)
                    return
                # finish the denominator (short tail after the last exp)
                acc = rp.tile([128, TOKG], F32, tag="acc", name="acc")
                with nc.allow_low_precision(
                        "fp16 partial + f32 combine for denominator"):
                    if racc1 is not None:
                        racc2 = rp.tile([128, TOKG], F16, tag="racc2",
                                        name="racc2")
                        nc.vector.tensor_reduce(
                            racc2, esw[:, :, racc1[1]:nvis],
                            mybir.AxisListType.X, mybir.AluOpType.add)
                        nc.vector.tensor_tensor(
                            acc, racc1[0], racc2, mybir.AluOpType.add)
                    else:
                        nc.vector.tensor_reduce(
                            acc, esw, mybir.AxisListType.X,
                            mybir.AluOpType.add)
                # PV accumulation (trim pure-causal zero prefix columns)
                for idx, (kc, (kind, mid)) in enumerate(vis):
                    qlo = max(0, 128 * kc - TOKG * g2)
                    nc.tensor.matmul(
                        o_ps[:, qlo:TOKG],
                        lhsT=vsb[:, b * NKC + kc, :],
                        rhs=esw[:, qlo:TOKG, idx],
                        start=(idx == 0), stop=(idx == nvis - 1))
                # partition reduce of the accumulated denominator (Pool)
                rsum = rp.tile([128, TOKG], F32, tag="rsum", name="rsum")
                nc.gpsimd.partition_all_reduce(
                    rsum, acc, channels=128, reduce_op=bass_isa.ReduceOp.add)
                rrec = rp.tile([128, TOKG], F32, tag="rrec", name="rrec")
                nc.vector.reciprocal(rrec, rsum)
                ow = op.tile([128, TOKG], F16, tag=f"outw{hh}", name="ow")
                nc.vector.tensor_tensor(ow, o_ps, rrec, mybir.AluOpType.mult)
                outTw.append(ow)

        def window_p3(n, tloc, outTw):
            """Output projection for one 128-token chunk of window n."""
            t0 = n * TOKG
            tch = t0 // 128 + tloc
            for et in range(D // 512):
                ps = ps_p3.tile([128, 512], F32, tag="p3", name="p3ps")
                for dc in range(HL):
                    nc.tensor.matmul(
                        ps,
                        lhsT=outTw[dc][:, tloc * 128:(tloc + 1) * 128],
                        rhs=wo[dc][:, et * 512:(et + 1) * 512],
                        start=(dc == 0), stop=(dc == HL - 1))
                pan = op.tile([128, 512], F16, tag="pan", name="pan",
                              bufs=6)
                if et % 2 == 0:
                    nc.scalar.copy(pan, ps)
                else:
                    nc.vector.tensor_scalar_add(pan, ps, 0.0)
                nc.sync.dma_start(
                    out=out_d[tch * 128:(tch + 1) * 128,
                              et * 512:(et + 1) * 512],
                    in_=pan)

        # --- prologue DMAs, just-in-time order (transfers serialize) ---
        x_r = xT_d[:].rearrange("(dc p) t -> p dc t", p=128)

        def x_slab(n, name):
            xs = xp.tile([128, NDC, TOKG], F16, tag="xslab", name=name)
            for dq in range(4):
                nc.sync.dma_start(
                    out=xs[:, dq * 8:(dq + 1) * 8, :],
                    in_=x_r[:, dq * 8:(dq + 1) * 8,
                            n * TOKG:(n + 1) * TOKG])
            return xs

        xsb0 = xp.tile([128, NDC, TOKG], F16, tag="xslab", name="xsb0")
        nc.sync.dma_start(out=xsb0[:, 0:4, :], in_=x_r[:, 0:4, 0:TOKG])
        nc.sync.dma_start(out=wsb[:, 0, 0:4, :], in_=w_d[0, :, 0:4, :])
        nc.sync.dma_start(out=xsb0[:, 4:8, :], in_=x_r[:, 4:8, 0:TOKG])
        for dw in range(1, 4):                # rest of w e=0 in quarters
            nc.sync.dma_start(out=wsb[:, 0, dw * 8:(dw + 1) * 8, :],
                              in_=w_d[0, :, dw * 8:(dw + 1) * 8, :])
        nc.sync.dma_start(out=wsb[:, 0, 4:8, :], in_=w_d[0, :, 4:8, :])
        for dq in range(1, 4):
            nc.sync.dma_start(out=xsb0[:, dq * 8:(dq + 1) * 8, :],
                              in_=x_r[:, dq * 8:(dq + 1) * 8, 0:TOKG])
        for half in range(2):                 # w e=1 in halves
            nc.sync.dma_start(out=wsb[:, 1, half * 16:(half + 1) * 16, :],
                              in_=w_d[1, :, half * 16:(half + 1) * 16, :])
        for e in range(2, NE):
            nc.sync.dma_start(out=wsb[:, e, :, :], in_=w_d[e])
        xpre = {0: xsb0, 1: x_slab(1, "xsb1")}
        nc.sync.dma_start(out=cos_sb, in_=cos_d[:])
        nc.sync.dma_start(out=sin_sb, in_=sin_d[:])
        nc.sync.dma_start(out=mask_sb,
                          in_=masks_d[:].rearrange("n p q -> p n q"))
        xpre[2] = x_slab(2, "xsb2")
        for dc in range(HL):
            nc.sync.dma_start(out=wo[dc], in_=wo_d[dc])
        xpre[3] = x_slab(3, "xsb3")

        def p1_phase(n, e, xsb_cur):
            """One eviction-unit of P1: e in 0..4 -> qkv row chunk;
            e == 5 -> both v token chunks."""
            t0 = n * TOKG
            if e < 5:
                ps = ps_p1.tile([128, TOKG], F32, tag="p1", name="p1ps")
                for dc in range(NDC):
                    nc.tensor.matmul(
                        ps,
                        lhsT=wsb[:, e, dc, :],
                        rhs=xsb_cur[:, dc, :],
                        start=(dc == 0), stop=(dc == NDC - 1))
                # fold softmax 1/sqrt(HD) into q rows at eviction
                nc.scalar.mul(qkvT[e][:, t0:t0 + TOKG], ps,
                              SCALE if e < HL else 1.0)
            else:
                for tch in range(TOKG // 128):  # v in [tok, hd] orientation
                    vp = ps_p1.tile([128, 128], F32, tag="p1", name="vps")
                    for dc in range(NDC):
                        nc.tensor.matmul(
                            vp,
                            lhsT=xsb_cur[:, dc, tch * 128:(tch + 1) * 128],
                            rhs=wsb[:, 5, dc, :],
                            start=(dc == 0), stop=(dc == NDC - 1))
                    nc.scalar.copy(vsb[:, t0 // 128 + tch, :], vp)

        def rope_one(n, e):
            """In-place RoPE of group n's row-chunk e (0..3 q heads, 4 k)."""
            g2 = n % NGB
            t0 = n * TOKG
            cs = cos_sb[:, g2 * TOKG:(g2 + 1) * TOKG]
            sn = sin_sb[:, g2 * TOKG:(g2 + 1) * TOKG]
            _rope_ops(nc, op, qkvT[e][:, t0:t0 + TOKG],
                      qkvT[e][:, t0:t0 + TOKG], cs, sn)

        xsb_cur = xsb0
        prevw = None        # (window_n, outTw) with pending pieces
        for n in range(NG):
            # prefetch next x slab (first few were issued in the prologue)
            if n + 1 < NG and n + 1 not in xpre:
                xpre[n + 1] = x_slab(n + 1, f"xsb{n + 1}")
            xsb_nxt = xpre.get(n + 1)

            last = n == NG - 1
            # last group: k and q first with inline rope, so the final
            # window's exp chains start as early as possible
            e_order = (4, 0, 1, 2, 3, 5) if last else (0, 1, 2, 3, 4, 5)
            outTw = []
            for step, e in enumerate(e_order):
                p1_phase(n, e, xsb_cur)
                if last and e != 5:
                    rope_one(n, e)
                # window n-1 pieces between P1 eviction units
                if prevw is not None and step < 6:
                    wn, wout = prevw
                    if step < 4:
                        window_head(wn, step, wout)
                    elif step == 4:
                        window_p3(wn, 0, wout)
                    else:
                        window_p3(wn, 1, wout)
            prevw = None
            xsb_cur = xsb_nxt if n + 1 < NG else None

            if not last:
                # RoPE in place for this group (k first: QK needs it)
                rope_one(n, 4)
                for hh in range(HL):
                    rope_one(n, hh)
            prevw = (n, outTw)

        # final window: heads + P3, nothing left to interleave
        wn, wout = prevw
        for hh in range(HL):
            window_head(wn, hh, wout, tail=True)
        window_p3(wn, 0, wout)
        window_p3(wn, 1, wout)

    nc.finalize()
    return nc, nmask


_CACHE = {}


def _get_nc(window: int):
    if window not in _CACHE:
        _CACHE[window] = build_nc(window)
    return _CACHE[window]


LAST_RESULTS = None


def kernel(x, w_qkv, w_o, window_size, _trace=False):
    window = int(window_size)
    nc, nmask = _get_nc(window)
    _, keys = _mask_plan(window)
    masks = _build_masks(window, keys)

    xT = np.ascontiguousarray(x.reshape(TOK, D).T).astype(F16NP)

    inv = 1.0 / (THETA ** (np.arange(0, HD, 2, dtype=np.float64) / HD))
    freqs = np.arange(T, dtype=np.float64)[:, None] * inv[None, :]  # [T, 64]
    cosH = np.repeat(np.cos(freqs).T, 2, axis=0).astype(F16NP)  # [128, T]
    sign = np.where(np.arange(HD) % 2 == 0, -1.0, 1.0)[:, None]
    sinH = (np.repeat(np.sin(freqs).T, 2, axis=0) * sign).astype(F16NP)

    in_maps = []
    for c in range(NCORES):
        wq = w_qkv[QROWS * c:QROWS * (c + 1)]
        wk = w_qkv[H * HD + HD * c: H * HD + HD * (c + 1)]
        wv = w_qkv[H * HD + G * HD + HD * c: H * HD + G * HD + HD * (c + 1)]
        wqkvT = np.concatenate([wq, wk, wv], axis=0).T  # [D, E] f32
        # e-major pack: [e_chunk, partition(=dc inner 128 rows), dc, 128]
        wE = np.ascontiguousarray(
            wqkvT.reshape(NDC, 128, NE, 128).transpose(2, 1, 0, 3)
        ).astype(F16NP)
        woT = np.ascontiguousarray(
            w_o[:, QROWS * c:QROWS * (c + 1)].T.reshape(HL, 128, D)
        ).astype(F16NP)
        in_maps.append({
            "xT": xT, "wE": wE, "woT": woT,
            "cosH": cosH, "sinH": sinH, "masks": masks.astype(F16NP),
        })

    from concourse.bass_utils import run_bass_kernel_spmd
    res = run_bass_kernel_spmd(nc, in_maps, core_ids=list(range(NCORES)),
                               trace=_trace)
    global LAST_RESULTS
    LAST_RESULTS = res
    acc = res.results[0]["out"].astype(np.float32)
    for c in range(1, NCORES):
        acc = acc + res.results[c]["out"].astype(np.float32)
    return acc.reshape(B, T, D)


# revision 31
# speedup vs baseline: 1.3568x; 1.1773x over previous
"""Trainium2 Bass kernel: fused QKV + RoPE + causal/windowed GQA attention + output proj.

Sharding: tensor-parallel by head across 8 cores. Core c owns Q-heads
4c..4c+3 and KV-group c, plus the 512 w_o columns for those heads. Each
core computes a full-shape fp16 partial of the final output; the host
sums the 8 partials. No device collectives.

Single fused pipeline: after P1 finishes the 256-token group
n = b*8 + g2, that group's K/V rows are final, so window (b, g2) of
attention is emitted staggered into the next group's P1 stream - P2/P3
instructions fill PE gaps inside P1's stream and vice versa.

The two big projections (P1, P3) run as fp8e4m3 DoubleRow matmuls
(0.5 PE cycles/row, 256-deep contraction) using an exact hi/lo split:
  a*b ~= a_hi*b_hi + a_hi*b_lo + a_lo*b_hi,   v_hi = fp8(v),
  v_lo = fp8(v - v_hi)  (weights pre-scaled x64 on host so residuals
  clear the fp8 subnormal floor; the x64 / x16 factors are folded into
  eviction scales for free).
3 terms x 0.5 cycles x half the instructions = 0.75x the PE time of
fp16 at ~0.2% error. Attention (QK/PV, contraction 128) stays fp16:
  P2: per (b, g2, head): ST[k, q] pair-tiles -> exp pairs into a window
      est buffer [128, 256, nvis]; diagonal chunks masked by DVE mult;
      denominator = staged DVE chunk-reduces + Pool partition_all_reduce
      (no rowsum matmuls); PV accumulates o_ps; DVE normalizes (x16)
      and splits the result into fp8 hi/lo for P3.
"""

import math
import sys
from contextlib import ExitStack

import numpy as np

sys.path.insert(0, "/opt/trn_rl_repo")

import ml_dtypes

F16NP = np.float16
E4NP = ml_dtypes.float8_e4m3

import concourse.bass as bass
import concourse.mybir as mybir
import concourse.tile as tile
from concourse import bacc
from concourse import bass_isa

F32 = mybir.dt.float32
F16 = mybir.dt.float16
F8 = mybir.dt.float8e4
DR = mybir.MatmulPerfMode.DoubleRow

B, T, D = 2, 2048, 4096
H, G, HD = 32, 8, 128
THETA = 10000.0
NCORES = 8
HL = H // NCORES            # 4 local q heads
TOK = B * T                 # 4096
QROWS = HL * HD             # 512 local q rows
E = QROWS + 2 * HD          # 768 local qkv rows
SCALE = 1.0 / math.sqrt(HD)
WSC = 64.0                  # host pre-scale on w_qkv / w_o (fp8 subnormals)
ASC = 16.0                  # device pre-scale on attn output before fp8 split

TOKG = 256                  # P1 token-group width == P2 query-group width
NG = TOK // TOKG            # 16 groups; group n = (b= n//8, g2= n%8)
NGB = T // TOKG             # 8 groups per batch
NDC = D // 128              # 32 contraction chunks
NDP = NDC // 2              # 16 DoubleRow chunk pairs
NE = E // 128               # 6 qkv row chunks (4 q, 1 k, 1 v)
NKC = T // 128              # 16 key chunks per batch


def _mask_plan(window: int):
    """Per (g2, kc): 'skip', 'full', or mask-key (i-j offset based).

    Chunks are always computed full-width (256 queries); masked chunks
    multiply by a {0,1} mask afterward, so the est buffer holds exact
    zeros outside the visible region.
    """
    plan = {}
    keys = {}
    for g in range(NGB):
        for kc in range(NKC):
            i_min, i_max = TOKG * g, TOKG * g + TOKG - 1
            j_min, j_max = 128 * kc, 128 * kc + 127
            if j_min > i_max or (i_min - j_max) >= window:
                plan[(g, kc)] = ("skip", None)
            elif j_max <= i_min and (i_max - j_min) < window:
                plan[(g, kc)] = ("full", None)
            else:
                key = TOKG * g - 128 * kc
                if key not in keys:
                    keys[key] = len(keys)
                plan[(g, kc)] = ("mask", keys[key])
    return plan, keys


def _build_masks(window: int, keys: dict) -> np.ndarray:
    n = max(1, len(keys))
    m = np.zeros((n, 128, TOKG), dtype=np.float32)
    for key, idx in keys.items():
        qq = np.arange(TOKG)[None, :]
        kk = np.arange(128)[:, None]
        diff = key + qq - kk          # i - j
        vis = (diff >= 0) & (diff < window)
        m[idx] = np.where(vis, 1.0, 0.0)
    return m


PAIRSWAP = [i ^ 1 for i in range(32)]


def _rope_ops(nc, pool, dst, src, cos_ap, sin_ap):
    """Interleaved-pair RoPE: dst = src*cos + pairswap(src)*signed_sin.

    cos_ap rows (2i, 2i+1) hold cos_i; sin_ap rows hold (-sin_i, +sin_i).
    src may alias dst (in-place).
    """
    W = dst.shape[-1]
    sw = pool.tile([128, W], F16, tag="rope_sw", name="rope_sw")
    tmp = pool.tile([128, W], F16, tag="rope_tmp", name="rope_tmp")
    qc = pool.tile([128, W], F16, tag="rope_qc", name="rope_qc")
    mult = mybir.AluOpType.mult
    nc.vector.stream_shuffle(sw, src, PAIRSWAP)
    nc.vector.tensor_tensor(tmp, sw, sin_ap, mult)
    nc.vector.tensor_tensor(qc, src, cos_ap, mult)
    nc.vector.tensor_tensor(dst, qc, tmp, mybir.AluOpType.add)


def build_nc(window: int):
    plan, keys = _mask_plan(window)
    nmask = max(1, len(keys))

    nc = bacc.Bacc()
    xh_d = nc.dram_tensor("xh", [D, TOK], F8, kind="ExternalInput")
    xl_d = nc.dram_tensor("xl", [D, TOK], F8, kind="ExternalInput")
    # host-packed e-major: [e_chunk, partition, dc, 128], pre-scaled x64
    wh_d = nc.dram_tensor("wh", [NE, 128, NDC, 128], F8, kind="ExternalInput")
    wl_d = nc.dram_tensor("wl", [NE, 128, NDC, 128], F8, kind="ExternalInput")
    # [partition, local_head_chunk, D], pre-scaled x64
    woh_d = nc.dram_tensor("woh", [128, HL, D], F8, kind="ExternalInput")
    wol_d = nc.dram_tensor("wol", [128, HL, D], F8, kind="ExternalInput")
    cos_d = nc.dram_tensor("cosH", [128, T], F16, kind="ExternalInput")
    sin_d = nc.dram_tensor("sinH", [128, T], F16, kind="ExternalInput")
    masks_d = nc.dram_tensor("masks", [nmask, 128, TOKG], F16, kind="ExternalInput")
    out_d = nc.dram_tensor("out", [TOK, D], F16, kind="ExternalOutput")

    with ExitStack() as octx:
        tc = octx.enter_context(tile.TileContext(nc))
        # persistent SBUF
        pers = octx.enter_context(tc.tile_pool(name="pers", bufs=1))
        wh = pers.tile([128, NE, NDC, 128], F8, name="wh")
        wl = pers.tile([128, NE, NDC, 128], F8, name="wl")
        qkvT = [pers.tile([128, TOK], F16, tag=f"qkv{e}", name=f"qkv{e}")
                for e in range(5)]               # 4 q heads + k
        ksb = qkvT[HL]
        vsb = pers.tile([128, TOK // 128, 128], F16, name="vsb")
        cos_sb = pers.tile([128, T], F16, name="cos_sb")
        sin_sb = pers.tile([128, T], F16, name="sin_sb")
        mask_sb = pers.tile([128, nmask, TOKG], F16, name="mask_sb")
        woh = pers.tile([128, HL, D], F8, name="woh")
        wol = pers.tile([128, HL, D], F8, name="wol")

        # working pools
        xp = octx.enter_context(tc.tile_pool(name="xp", bufs=2))
        ep = octx.enter_context(tc.tile_pool(name="ep", bufs=2))
        rp = octx.enter_context(tc.tile_pool(name="rp", bufs=2))
        op = octx.enter_context(tc.tile_pool(name="op", bufs=2))
        ps_p1 = octx.enter_context(tc.tile_pool(name="ps1", bufs=2, space="PSUM"))
        ps_st = octx.enter_context(tc.tile_pool(name="ps_st", bufs=2, space="PSUM"))
        ps_o = octx.enter_context(tc.tile_pool(name="ps_o", bufs=2, space="PSUM"))
        ps_p3 = octx.enter_context(tc.tile_pool(name="ps3", bufs=2, space="PSUM"))

        def window_head(n, hh, wtiles, tail=False):
            """One head's QK->exp->PV->denominator->normalize chain.

            tail=True (final window, P1 finished): steal idle ps_p1 banks
            as extra score buffers and stage the denominator reduce in
            more, smaller pieces (the tail has no P1 to hide latency).
            """
            b, g2 = divmod(n, NGB)
            t0 = n * TOKG
            vis = [(kc, plan[(g2, kc)]) for kc in range(NKC)
                   if plan[(g2, kc)][0] != "skip"]
            nvis = len(vis)
            if hh == 0:
                wtiles["h"] = op.tile([128, HL, TOKG], F8, tag="owh",
                                      name="owh")
                wtiles["l"] = op.tile([128, HL, TOKG], F8, tag="owl",
                                      name="owl")
            esw = ep.tile([128, TOKG, nvis], F16, tag="esw", name="esw",
                          padded_shape=[128, TOKG, NKC])
            o_ps = ps_o.tile([128, TOKG], F32, tag="o", name="o_ps")
            qtile = qkvT[hh][:, t0:t0 + TOKG]
            racc1 = None
            mids_at = ([nvis // 4 * 2, nvis - 2] if tail and nvis > 6
                       else [nvis - 2])
            # QK + exp in pairs sharing one PSUM bank
            i = 0
            while i < nvis:
                npair = min(2, nvis - i)
                stp = ps_p1 if (tail and (i // 2) % 2 == 1) else ps_st
                st = stp.tile([128, 2, TOKG], F32,
                              tag="p1" if stp is ps_p1 else "st",
                              name="st")
                for j in range(npair):
                    kc = vis[i + j][0]
                    nc.tensor.matmul(
                        st[:, j, :],
                        lhsT=ksb[:, b * T + kc * 128:b * T + (kc + 1) * 128],
                        rhs=qtile,
                        start=True, stop=True)
                nc.scalar.activation(
                    esw[:, :, i:i + npair].rearrange("p q s -> p s q"),
                    st[:, 0:npair, :],
                    mybir.ActivationFunctionType.Exp)
                # mask chunks in this pair (diagonal / window edges)
                for j in range(npair):
                    kind, mid = vis[i + j][1]
                    if kind == "mask":
                        sl = esw[:, :, i + j]
                        nc.vector.tensor_tensor(
                            sl, sl, mask_sb[:, mid, :],
                            mybir.AluOpType.mult)
                i += npair
                # staged partial denominators keep the post-exp chain short
                if i in mids_at and nvis > 2:
                    part = rp.tile([128, TOKG], F16, tag="racc1",
                                   name="racc1")
                    lo = 0 if racc1 is None else racc1[1]
                    with nc.allow_low_precision(
                            "est chunk-reduce; denominator finished in f32"):
                        nc.vector.tensor_reduce(
                            part, esw[:, :, lo:i], mybir.AxisListType.X,
                            mybir.AluOpType.add)
                    if racc1 is None:
                        racc1 = (part, i)
                    else:
                        comb = rp.tile([128, TOKG], F16, tag="rcomb",
                                       name="racc1c")
                        nc.vector.tensor_tensor(
                            comb, racc1[0], part, mybir.AluOpType.add)
                        racc1 = (comb, i)
            # PV accumulation (trim pure-causal zero prefix columns)
            for idx, (kc, (kind, mid)) in enumerate(vis):
                qlo = max(0, 128 * kc - TOKG * g2)
                nc.tensor.matmul(
                    o_ps[:, qlo:TOKG],
                    lhsT=vsb[:, b * NKC + kc, :],
                    rhs=esw[:, qlo:TOKG, idx],
                    start=(idx == 0), stop=(idx == nvis - 1))
            # finish the denominator (short tail after the last exp)
            acc = rp.tile([128, TOKG], F32, tag="acc", name="acc")
            with nc.allow_low_precision(
                    "fp16 partial + f32 combine for denominator"):
                if racc1 is not None:
                    racc2 = rp.tile([128, TOKG], F16, tag="racc2",
                                    name="racc2")
                    nc.vector.tensor_reduce(
                        racc2, esw[:, :, racc1[1]:nvis],
                        mybir.AxisListType.X, mybir.AluOpType.add)
                    nc.vector.tensor_tensor(
                        acc, racc1[0], racc2, mybir.AluOpType.add)
                else:
                    nc.vector.tensor_reduce(
                        acc, esw, mybir.AxisListType.X,
                        mybir.AluOpType.add)
            # partition reduce of the accumulated denominator (Pool)
            rsum = rp.tile([128, TOKG], F32, tag="rsum", name="rsum")
            nc.gpsimd.partition_all_reduce(
                rsum, acc, channels=128, reduce_op=bass_isa.ReduceOp.add)
            rrec = rp.tile([128, TOKG], F32, tag="rrec", name="rrec")
            nc.vector.reciprocal(rrec, rsum)
            # normalize (xASC) and split fp8 hi/lo for the DoubleRow P3
            at16 = rp.tile([128, TOKG], F16, tag="at16", name="at16")
            nc.vector.scalar_tensor_tensor(
                at16, o_ps, ASC, rrec,
                mybir.AluOpType.mult, mybir.AluOpType.mult)
            nc.scalar.copy(wtiles["h"][:, hh, :], at16)
            nc.vector.tensor_tensor(
                wtiles["l"][:, hh, :], at16, wtiles["h"][:, hh, :],
                mybir.AluOpType.subtract)

        def window_p3(n, tloc, wtiles):
            """Output projection for one 128-token chunk of window n."""
            t0 = n * TOKG
            tch = t0 // 128 + tloc
            owh_t, owl_t = wtiles["h"], wtiles["l"]
            tsl = slice(tloc * 128, (tloc + 1) * 128)
            for et in range(D // 512):
                ps = ps_p3.tile([128, 512], F32, tag="p3", name="p3ps")
                esl = slice(et * 512, (et + 1) * 512)
                for dcp in range(HL // 2):
                    hsl = slice(2 * dcp, 2 * dcp + 2)
                    for tnum, (lo, ro) in enumerate(
                            ((owh_t, woh), (owh_t, wol), (owl_t, woh))):
                        nc.tensor.matmul(
                            ps,
                            lhsT=lo[:, hsl, tsl],
                            rhs=ro[:, hsl, esl],
                            start=(dcp == 0 and tnum == 0),
                            stop=(dcp == HL // 2 - 1 and tnum == 2),
                            perf_mode=DR)
                pan = op.tile([128, 512], F16, tag="pan", name="pan",
                              bufs=5)
                if et % 2 == 0:
                    nc.scalar.mul(pan, ps, 1.0 / (WSC * ASC))
                else:
                    nc.vector.tensor_scalar_mul(pan, ps, 1.0 / (WSC * ASC))
                nc.sync.dma_start(
                    out=out_d[tch * 128:(tch + 1) * 128, esl],
                    in_=pan)

        def p1_phase(n, e, xhc, xlc):
            """One eviction-unit of P1 as 3-term hi/lo DoubleRow fp8:
            e in 0..4 -> qkv row chunk; e == 5 -> both v token chunks."""
            t0 = n * TOKG
            if e < 5:
                ps = ps_p1.tile([128, TOKG], F32, tag="p1", name="p1ps")
                for tnum, (lw, rx) in enumerate(
                        ((wh, xhc), (wh, xlc), (wl, xhc))):
                    for dp in range(NDP):
                        nc.tensor.matmul(
                            ps,
                            lhsT=lw[:, e, 2 * dp:2 * dp + 2, :],
                            rhs=rx[:, 2 * dp:2 * dp + 2, :],
                            start=(tnum == 0 and dp == 0),
                            stop=(tnum == 2 and dp == NDP - 1),
                            perf_mode=DR)
                # fold softmax 1/sqrt(HD) and the 1/WSC w-prescale into
                # the eviction
                nc.scalar.mul(qkvT[e][:, t0:t0 + TOKG], ps,
                              (SCALE if e < HL else 1.0) / WSC)
            else:
                for tch in range(TOKG // 128):  # v in [tok, hd] orientation
                    vp = ps_p1.tile([128, 128], F32, tag="p1", name="vps")
                    ts = slice(tch * 128, (tch + 1) * 128)
                    for tnum, (lx, rw) in enumerate(
                            ((xhc, wh), (xhc, wl), (xlc, wh))):
                        for dp in range(NDP):
                            nc.tensor.matmul(
                                vp,
                                lhsT=lx[:, 2 * dp:2 * dp + 2, ts],
                                rhs=rw[:, 5, 2 * dp:2 * dp + 2, :],
                                start=(tnum == 0 and dp == 0),
                                stop=(tnum == 2 and dp == NDP - 1),
                                perf_mode=DR)
                    nc.scalar.mul(vsb[:, t0 // 128 + tch, :], vp, 1.0 / WSC)

        def rope_one(n, e):
            """In-place RoPE of group n's row-chunk e (0..3 q heads, 4 k)."""
            g2 = n % NGB
            t0 = n * TOKG
            cs = cos_sb[:, g2 * TOKG:(g2 + 1) * TOKG]
            sn = sin_sb[:, g2 * TOKG:(g2 + 1) * TOKG]
            _rope_ops(nc, op, qkvT[e][:, t0:t0 + TOKG],
                      qkvT[e][:, t0:t0 + TOKG], cs, sn)

        # --- prologue DMAs, just-in-time order (transfers serialize) ---
        xh_r = xh_d[:].rearrange("(dc p) t -> p dc t", p=128)
        xl_r = xl_d[:].rearrange("(dc p) t -> p dc t", p=128)

        def x_slab(n, suffix=""):
            xhc = xp.tile([128, NDC, TOKG], F8, tag="xh", name=f"xh{suffix}")
            xlc = xp.tile([128, NDC, TOKG], F8, tag="xl", name=f"xl{suffix}")
            sl = slice(n * TOKG, (n + 1) * TOKG)
            for dq in range(2):
                nc.sync.dma_start(out=xhc[:, dq * 16:(dq + 1) * 16, :],
                                  in_=xh_r[:, dq * 16:(dq + 1) * 16, sl])
            for dq in range(2):
                nc.sync.dma_start(out=xlc[:, dq * 16:(dq + 1) * 16, :],
                                  in_=xl_r[:, dq * 16:(dq + 1) * 16, sl])
            return (xhc, xlc)

        # group 0 pieces in consumption order: term1 (wh,xh) first
        xh0 = xp.tile([128, NDC, TOKG], F8, tag="xh", name="xh0")
        xl0 = xp.tile([128, NDC, TOKG], F8, tag="xl", name="xl0")
        for dq in range(4):
            nc.sync.dma_start(out=xh0[:, dq * 8:(dq + 1) * 8, :],
                              in_=xh_r[:, dq * 8:(dq + 1) * 8, 0:TOKG])
            if dq == 0:
                nc.sync.dma_start(out=wh[:, 0, 0:16, :],
                                  in_=wh_d[0, :, 0:16, :])
            elif dq == 1:
                nc.sync.dma_start(out=wh[:, 0, 16:32, :],
                                  in_=wh_d[0, :, 16:32, :])
        for dq in range(4):
            nc.sync.dma_start(out=xl0[:, dq * 8:(dq + 1) * 8, :],
                              in_=xl_r[:, dq * 8:(dq + 1) * 8, 0:TOKG])
        nc.sync.dma_start(out=wl[:, 0, :, :], in_=wl_d[0])
        for e in range(1, NE):
            nc.sync.dma_start(out=wh[:, e, :, :], in_=wh_d[e])
            nc.sync.dma_start(out=wl[:, e, :, :], in_=wl_d[e])
        xpre = {0: (xh0, xl0), 1: x_slab(1, "1")}
        nc.sync.dma_start(out=cos_sb, in_=cos_d[:])
        nc.sync.dma_start(out=sin_sb, in_=sin_d[:])
        nc.sync.dma_start(out=mask_sb,
                          in_=masks_d[:].rearrange("n p q -> p n q"))
        xpre[2] = x_slab(2, "2")
        nc.sync.dma_start(out=woh, in_=woh_d[:])
        nc.sync.dma_start(out=wol, in_=wol_d[:])
        xpre[3] = x_slab(3, "3")

        xsb_cur = xpre[0]
        prevw = None        # (window_n, wtiles) with pending pieces
        for n in range(NG):
            # prefetch next x slab (first few were issued in the prologue)
            if n + 1 < NG and n + 1 not in xpre:
                xpre[n + 1] = x_slab(n + 1, str(n + 1))
            xsb_nxt = xpre.get(n + 1)

            last = n == NG - 1
            # last group: k and q first with inline rope, so the final
            # window's exp chains start as early as possible
            e_order = (4, 0, 1, 2, 3, 5) if last else (0, 1, 2, 3, 4, 5)
            wtiles = {}
            for step, e in enumerate(e_order):
                p1_phase(n, e, *xsb_cur)
                if last and e != 5:
                    rope_one(n, e)
                # window n-1 pieces between P1 eviction units
                if prevw is not None and step < 6:
                    wn, wout = prevw
                    if step < 4:
                        window_head(wn, step, wout)
                    elif step == 4:
                        window_p3(wn, 0, wout)
                    else:
                        window_p3(wn, 1, wout)
            prevw = None
            xsb_cur = xsb_nxt

            if not last:
                # RoPE in place for this group (k first: QK needs it)
                rope_one(n, 4)
                for hh in range(HL):
                    rope_one(n, hh)
            prevw = (n, wtiles)

        # final window: heads + P3, nothing left to interleave
        wn, wout = prevw
        for hh in range(HL):
            window_head(wn, hh, wout, tail=True)
        window_p3(wn, 0, wout)
        window_p3(wn, 1, wout)

    nc.finalize()
    return nc, nmask


_CACHE = {}


def _get_nc(window: int):
    if window not in _CACHE:
        _CACHE[window] = build_nc(window)
    return _CACHE[window]


LAST_RESULTS = None


def _hilo(a32):
    hi = a32.astype(E4NP)
    lo = (a32 - hi.astype(np.float32)).astype(E4NP)
    return hi, lo


def kernel(x, w_qkv, w_o, window_size, _trace=False):
    window = int(window_size)
    nc, nmask = _get_nc(window)
    _, keys = _mask_plan(window)
    masks = _build_masks(window, keys)

    xT = np.ascontiguousarray(x.reshape(TOK, D).T).astype(np.float32)
    xh, xl = _hilo(xT)

    inv = 1.0 / (THETA ** (np.arange(0, HD, 2, dtype=np.float64) / HD))
    freqs = np.arange(T, dtype=np.float64)[:, None] * inv[None, :]  # [T, 64]
    cosH = np.repeat(np.cos(freqs).T, 2, axis=0).astype(F16NP)  # [128, T]
    sign = np.where(np.arange(HD) % 2 == 0, -1.0, 1.0)[:, None]
    sinH = (np.repeat(np.sin(freqs).T, 2, axis=0) * sign).astype(F16NP)

    in_maps = []
    for c in range(NCORES):
        wq = w_qkv[QROWS * c:QROWS * (c + 1)]
        wk = w_qkv[H * HD + HD * c: H * HD + HD * (c + 1)]
        wv = w_qkv[H * HD + G * HD + HD * c: H * HD + G * HD + HD * (c + 1)]
        wqkvT = np.concatenate([wq, wk, wv], axis=0).T * WSC  # [D, E] f32
        # e-major pack: [e_chunk, partition(=dc inner 128 rows), dc, 128]
        wE = np.ascontiguousarray(
            wqkvT.reshape(NDC, 128, NE, 128).transpose(2, 1, 0, 3)
        ).astype(np.float32)
        wh_, wl_ = _hilo(wE)
        woT = np.ascontiguousarray(
            (w_o[:, QROWS * c:QROWS * (c + 1)] * WSC).T.reshape(HL, 128, D)
            .transpose(1, 0, 2)).astype(np.float32)  # [128, HL, D]
        woh_, wol_ = _hilo(woT)
        in_maps.append({
            "xh": xh, "xl": xl, "wh": wh_, "wl": wl_,
            "woh": woh_, "wol": wol_,
            "cosH": cosH, "sinH": sinH, "masks": masks.astype(F16NP),
        })

    from concourse.bass_utils import run_bass_kernel_spmd
    res = run_bass_kernel_spmd(nc, in_maps, core_ids=list(range(NCORES)),
                               trace=_trace)
    global LAST_RESULTS
    LAST_RESULTS = res
    acc = res.results[0]["out"].astype(np.float32)
    for c in range(1, NCORES):
        acc = acc + res.results[c]["out"].astype(np.float32)
    return acc.reshape(B, T, D)


# revision 32
# speedup vs baseline: 1.3775x; 1.0153x over previous
"""Trainium2 Bass kernel: fused QKV + RoPE + causal/windowed GQA attention + output proj.

Sharding: tensor-parallel by head across 8 cores. Core c owns Q-heads
4c..4c+3 and KV-group c, plus the 512 w_o columns for those heads. Each
core computes a full-shape fp16 partial of the final output; the host
sums the 8 partials. No device collectives.

Single fused pipeline: after P1 finishes the 256-token group
n = b*8 + g2, that group's K/V rows are final, so window (b, g2) of
attention is emitted staggered into the next group's P1 stream - P2/P3
instructions fill PE gaps inside P1's stream and vice versa.

The two big projections (P1, P3) run as fp8e4m3 DoubleRow matmuls
(0.5 PE cycles/row, 256-deep contraction) using an exact hi/lo split:
  a*b ~= a_hi*b_hi + a_hi*b_lo + a_lo*b_hi,   v_hi = fp8(v),
  v_lo = fp8(v - v_hi)  (weights pre-scaled x64 on host so residuals
  clear the fp8 subnormal floor; the x64 / x16 factors are folded into
  eviction scales for free).
3 terms x 0.5 cycles x half the instructions = 0.75x the PE time of
fp16 at ~0.2% error. Attention (QK/PV, contraction 128) stays fp16:
  P2: per (b, g2, head): ST[k, q] pair-tiles -> exp pairs into a window
      est buffer [128, 256, nvis]; diagonal chunks masked by DVE mult;
      denominator = staged DVE chunk-reduces + Pool partition_all_reduce
      (no rowsum matmuls); PV accumulates o_ps; DVE normalizes (x16)
      and splits the result into fp8 hi/lo for P3.
"""

import math
import sys
from contextlib import ExitStack

import numpy as np

sys.path.insert(0, "/opt/trn_rl_repo")

import ml_dtypes

F16NP = np.float16
E4NP = ml_dtypes.float8_e4m3

import concourse.bass as bass
import concourse.mybir as mybir
import concourse.tile as tile
from concourse import bacc
from concourse import bass_isa

F32 = mybir.dt.float32
F16 = mybir.dt.float16
F8 = mybir.dt.float8e4
DR = mybir.MatmulPerfMode.DoubleRow

B, T, D = 2, 2048, 4096
H, G, HD = 32, 8, 128
THETA = 10000.0
NCORES = 8
HL = H // NCORES            # 4 local q heads
TOK = B * T                 # 4096
QROWS = HL * HD             # 512 local q rows
E = QROWS + 2 * HD          # 768 local qkv rows
SCALE = 1.0 / math.sqrt(HD)
WSC = 64.0                  # host pre-scale on w_qkv / w_o (fp8 subnormals)
ASC = 16.0                  # device pre-scale on attn output before fp8 split

TOKG = 256                  # P1 token-group width == P2 query-group width
NG = TOK // TOKG            # 16 groups; group n = (b= n//8, g2= n%8)
NGB = T // TOKG             # 8 groups per batch
NDC = D // 128              # 32 contraction chunks
NDP = NDC // 2              # 16 DoubleRow chunk pairs
NE = E // 128               # 6 qkv row chunks (4 q, 1 k, 1 v)
NKC = T // 128              # 16 key chunks per batch


def _mask_plan(window: int):
    """Per (g2, kc): 'skip', 'full', or mask-key (i-j offset based).

    Chunks are always computed full-width (256 queries); masked chunks
    multiply by a {0,1} mask afterward, so the est buffer holds exact
    zeros outside the visible region.
    """
    plan = {}
    keys = {}
    for g in range(NGB):
        for kc in range(NKC):
            i_min, i_max = TOKG * g, TOKG * g + TOKG - 1
            j_min, j_max = 128 * kc, 128 * kc + 127
            if j_min > i_max or (i_min - j_max) >= window:
                plan[(g, kc)] = ("skip", None)
            elif j_max <= i_min and (i_max - j_min) < window:
                plan[(g, kc)] = ("full", None)
            else:
                key = TOKG * g - 128 * kc
                if key not in keys:
                    keys[key] = len(keys)
                plan[(g, kc)] = ("mask", keys[key])
    return plan, keys


def _build_masks(window: int, keys: dict) -> np.ndarray:
    n = max(1, len(keys))
    m = np.zeros((n, 128, TOKG), dtype=np.float32)
    for key, idx in keys.items():
        qq = np.arange(TOKG)[None, :]
        kk = np.arange(128)[:, None]
        diff = key + qq - kk          # i - j
        vis = (diff >= 0) & (diff < window)
        m[idx] = np.where(vis, 1.0, 0.0)
    return m


PAIRSWAP = [i ^ 1 for i in range(32)]


def _rope_ops(nc, pool, dst, src, cos_ap, sin_ap):
    """Interleaved-pair RoPE: dst = src*cos + pairswap(src)*signed_sin.

    cos_ap rows (2i, 2i+1) hold cos_i; sin_ap rows hold (-sin_i, +sin_i).
    src may alias dst (in-place).
    """
    W = dst.shape[-1]
    sw = pool.tile([128, W], F16, tag="rope_sw", name="rope_sw")
    tmp = pool.tile([128, W], F16, tag="rope_tmp", name="rope_tmp")
    qc = pool.tile([128, W], F16, tag="rope_qc", name="rope_qc")
    mult = mybir.AluOpType.mult
    nc.vector.stream_shuffle(sw, src, PAIRSWAP)
    nc.vector.tensor_tensor(tmp, sw, sin_ap, mult)
    nc.vector.tensor_tensor(qc, src, cos_ap, mult)
    nc.vector.tensor_tensor(dst, qc, tmp, mybir.AluOpType.add)


def build_nc(window: int):
    plan, keys = _mask_plan(window)
    nmask = max(1, len(keys))

    nc = bacc.Bacc()
    # hi/lo planes interleaved per dc chunk so DMA runs stay >= 512B
    x8_d = nc.dram_tensor("x8", [NG, 128, NDC, 2, TOKG], F8,
                          kind="ExternalInput")
    # host-packed e-major: [e_chunk, partition, dc, 128], pre-scaled x64
    wh_d = nc.dram_tensor("wh", [NE, 128, NDC, 128], F8, kind="ExternalInput")
    wl_d = nc.dram_tensor("wl", [NE, 128, NDC, 128], F8, kind="ExternalInput")
    # [partition, local_head_chunk, D], pre-scaled x64
    woh_d = nc.dram_tensor("woh", [128, HL, D], F8, kind="ExternalInput")
    wol_d = nc.dram_tensor("wol", [128, HL, D], F8, kind="ExternalInput")
    cos_d = nc.dram_tensor("cosH", [128, T], F16, kind="ExternalInput")
    sin_d = nc.dram_tensor("sinH", [128, T], F16, kind="ExternalInput")
    masks_d = nc.dram_tensor("masks", [nmask, 128, TOKG], F16, kind="ExternalInput")
    out_d = nc.dram_tensor("out", [TOK, D], F16, kind="ExternalOutput")

    with ExitStack() as octx:
        tc = octx.enter_context(tile.TileContext(nc))
        # persistent SBUF
        pers = octx.enter_context(tc.tile_pool(name="pers", bufs=1))
        wh = pers.tile([128, NE, NDC, 128], F8, name="wh")
        wl = pers.tile([128, NE, NDC, 128], F8, name="wl")
        qkvT = [pers.tile([128, TOK], F16, tag=f"qkv{e}", name=f"qkv{e}")
                for e in range(5)]               # 4 q heads + k
        ksb = qkvT[HL]
        vsb = pers.tile([128, TOK // 128, 128], F16, name="vsb")
        cos_sb = pers.tile([128, T], F16, name="cos_sb")
        sin_sb = pers.tile([128, T], F16, name="sin_sb")
        mask_sb = pers.tile([128, nmask, TOKG], F16, name="mask_sb")
        woh = pers.tile([128, HL, D], F8, name="woh")
        wol = pers.tile([128, HL, D], F8, name="wol")

        # working pools
        xp = octx.enter_context(tc.tile_pool(name="xp", bufs=2))
        ep = octx.enter_context(tc.tile_pool(name="ep", bufs=2))
        rp = octx.enter_context(tc.tile_pool(name="rp", bufs=2))
        op = octx.enter_context(tc.tile_pool(name="op", bufs=2))
        ps_p1 = octx.enter_context(tc.tile_pool(name="ps1", bufs=2, space="PSUM"))
        ps_st = octx.enter_context(tc.tile_pool(name="ps_st", bufs=2, space="PSUM"))
        ps_o = octx.enter_context(tc.tile_pool(name="ps_o", bufs=2, space="PSUM"))
        ps_p3 = octx.enter_context(tc.tile_pool(name="ps3", bufs=2, space="PSUM"))

        def window_head(n, hh, wtiles, tail=False):
            """One head's QK->exp->PV->denominator->normalize chain.

            tail=True (final window, P1 finished): steal idle ps_p1 banks
            as extra score buffers and stage the denominator reduce in
            more, smaller pieces (the tail has no P1 to hide latency).
            """
            b, g2 = divmod(n, NGB)
            t0 = n * TOKG
            vis = [(kc, plan[(g2, kc)]) for kc in range(NKC)
                   if plan[(g2, kc)][0] != "skip"]
            nvis = len(vis)
            if hh == 0:
                wtiles["h"] = op.tile([128, HL, TOKG], F8, tag="owh",
                                      name="owh")
                wtiles["l"] = op.tile([128, HL, TOKG], F8, tag="owl",
                                      name="owl")
            esw = ep.tile([128, TOKG, nvis], F16, tag="esw", name="esw",
                          padded_shape=[128, TOKG, NKC])
            o_ps = ps_o.tile([128, TOKG], F32, tag="o", name="o_ps")
            qtile = qkvT[hh][:, t0:t0 + TOKG]
            racc1 = None
            mids_at = ([nvis // 4 * 2, nvis - 2] if tail and nvis > 6
                       else [nvis - 2])
            # QK + exp in pairs sharing one PSUM bank
            i = 0
            while i < nvis:
                npair = min(2, nvis - i)
                stp = ps_p1 if (tail and (i // 2) % 2 == 1) else ps_st
                st = stp.tile([128, 2, TOKG], F32,
                              tag="p1" if stp is ps_p1 else "st",
                              name="st")
                for j in range(npair):
                    kc = vis[i + j][0]
                    nc.tensor.matmul(
                        st[:, j, :],
                        lhsT=ksb[:, b * T + kc * 128:b * T + (kc + 1) * 128],
                        rhs=qtile,
                        start=True, stop=True)
                nc.scalar.activation(
                    esw[:, :, i:i + npair].rearrange("p q s -> p s q"),
                    st[:, 0:npair, :],
                    mybir.ActivationFunctionType.Exp)
                # mask chunks in this pair (diagonal / window edges)
                for j in range(npair):
                    kind, mid = vis[i + j][1]
                    if kind == "mask":
                        sl = esw[:, :, i + j]
                        nc.vector.tensor_tensor(
                            sl, sl, mask_sb[:, mid, :],
                            mybir.AluOpType.mult)
                i += npair
                # staged partial denominators keep the post-exp chain short
                if i in mids_at and nvis > 2:
                    part = rp.tile([128, TOKG], F16, tag="racc1",
                                   name="racc1")
                    lo = 0 if racc1 is None else racc1[1]
                    with nc.allow_low_precision(
                            "est chunk-reduce; denominator finished in f32"):
                        nc.vector.tensor_reduce(
                            part, esw[:, :, lo:i], mybir.AxisListType.X,
                            mybir.AluOpType.add)
                    if racc1 is None:
                        racc1 = (part, i)
                    else:
                        comb = rp.tile([128, TOKG], F16, tag="rcomb",
                                       name="racc1c")
                        nc.vector.tensor_tensor(
                            comb, racc1[0], part, mybir.AluOpType.add)
                        racc1 = (comb, i)
            # PV accumulation (trim pure-causal zero prefix columns)
            for idx, (kc, (kind, mid)) in enumerate(vis):
                qlo = max(0, 128 * kc - TOKG * g2)
                nc.tensor.matmul(
                    o_ps[:, qlo:TOKG],
                    lhsT=vsb[:, b * NKC + kc, :],
                    rhs=esw[:, qlo:TOKG, idx],
                    start=(idx == 0), stop=(idx == nvis - 1))
            # finish the denominator (short tail after the last exp)
            acc = rp.tile([128, TOKG], F32, tag="acc", name="acc")
            with nc.allow_low_precision(
                    "fp16 partial + f32 combine for denominator"):
                if racc1 is not None:
                    racc2 = rp.tile([128, TOKG], F16, tag="racc2",
                                    name="racc2")
                    nc.vector.tensor_reduce(
                        racc2, esw[:, :, racc1[1]:nvis],
                        mybir.AxisListType.X, mybir.AluOpType.add)
                    nc.vector.tensor_tensor(
                        acc, racc1[0], racc2, mybir.AluOpType.add)
                else:
                    nc.vector.tensor_reduce(
                        acc, esw, mybir.AxisListType.X,
                        mybir.AluOpType.add)
            # partition reduce of the accumulated denominator (Pool)
            rsum = rp.tile([128, TOKG], F32, tag="rsum", name="rsum")
            nc.gpsimd.partition_all_reduce(
                rsum, acc, channels=128, reduce_op=bass_isa.ReduceOp.add)
            rrec = rp.tile([128, TOKG], F32, tag="rrec", name="rrec")
            nc.vector.reciprocal(rrec, rsum)
            # normalize (xASC) and split fp8 hi/lo for the DoubleRow P3
            at16 = rp.tile([128, TOKG], F16, tag="at16", name="at16")
            nc.vector.scalar_tensor_tensor(
                at16, o_ps, ASC, rrec,
                mybir.AluOpType.mult, mybir.AluOpType.mult)
            nc.scalar.copy(wtiles["h"][:, hh, :], at16)
            nc.vector.tensor_tensor(
                wtiles["l"][:, hh, :], at16, wtiles["h"][:, hh, :],
                mybir.AluOpType.subtract)

        def window_p3(n, tloc, wtiles):
            """Output projection for one 128-token chunk of window n."""
            t0 = n * TOKG
            tch = t0 // 128 + tloc
            owh_t, owl_t = wtiles["h"], wtiles["l"]
            tsl = slice(tloc * 128, (tloc + 1) * 128)
            for et in range(D // 512):
                ps = ps_p3.tile([128, 512], F32, tag="p3", name="p3ps")
                esl = slice(et * 512, (et + 1) * 512)
                for dcp in range(HL // 2):
                    hsl = slice(2 * dcp, 2 * dcp + 2)
                    for tnum, (lo, ro) in enumerate(
                            ((owh_t, woh), (owh_t, wol), (owl_t, woh))):
                        nc.tensor.matmul(
                            ps,
                            lhsT=lo[:, hsl, tsl],
                            rhs=ro[:, hsl, esl],
                            start=(dcp == 0 and tnum == 0),
                            stop=(dcp == HL // 2 - 1 and tnum == 2),
                            perf_mode=DR)
                pan = op.tile([128, 512], F16, tag="pan", name="pan",
                              bufs=5)
                if et % 2 == 0:
                    nc.scalar.mul(pan, ps, 1.0 / (WSC * ASC))
                else:
                    nc.vector.tensor_scalar_mul(pan, ps, 1.0 / (WSC * ASC))
                nc.sync.dma_start(
                    out=out_d[tch * 128:(tch + 1) * 128, esl],
                    in_=pan)

        def p1_phase(n, e, xsc):
            """One eviction-unit of P1 as 3-term hi/lo DoubleRow fp8:
            e in 0..4 -> qkv row chunk; e == 5 -> both v token chunks."""
            t0 = n * TOKG
            if e < 5:
                ps = ps_p1.tile([128, TOKG], F32, tag="p1", name="p1ps")
                for tnum, (lw, xs) in enumerate(
                        ((wh, 0), (wh, 1), (wl, 0))):
                    for dp in range(NDP):
                        nc.tensor.matmul(
                            ps,
                            lhsT=lw[:, e, 2 * dp:2 * dp + 2, :],
                            rhs=xsc[:, 2 * dp:2 * dp + 2, xs, :],
                            start=(tnum == 0 and dp == 0),
                            stop=(tnum == 2 and dp == NDP - 1),
                            perf_mode=DR)
                # fold softmax 1/sqrt(HD) and the 1/WSC w-prescale into
                # the eviction
                nc.scalar.mul(qkvT[e][:, t0:t0 + TOKG], ps,
                              (SCALE if e < HL else 1.0) / WSC)
            else:
                for tch in range(TOKG // 128):  # v in [tok, hd] orientation
                    vp = ps_p1.tile([128, 128], F32, tag="p1", name="vps")
                    ts = slice(tch * 128, (tch + 1) * 128)
                    for tnum, (xs, rw) in enumerate(
                            ((0, wh), (0, wl), (1, wh))):
                        for dp in range(NDP):
                            nc.tensor.matmul(
                                vp,
                                lhsT=xsc[:, 2 * dp:2 * dp + 2, xs, ts],
                                rhs=rw[:, 5, 2 * dp:2 * dp + 2, :],
                                start=(tnum == 0 and dp == 0),
                                stop=(tnum == 2 and dp == NDP - 1),
                                perf_mode=DR)
                    nc.scalar.mul(vsb[:, t0 // 128 + tch, :], vp, 1.0 / WSC)

        def rope_one(n, e):
            """In-place RoPE of group n's row-chunk e (0..3 q heads, 4 k)."""
            g2 = n % NGB
            t0 = n * TOKG
            cs = cos_sb[:, g2 * TOKG:(g2 + 1) * TOKG]
            sn = sin_sb[:, g2 * TOKG:(g2 + 1) * TOKG]
            _rope_ops(nc, op, qkvT[e][:, t0:t0 + TOKG],
                      qkvT[e][:, t0:t0 + TOKG], cs, sn)

        # --- prologue DMAs, just-in-time order (transfers serialize) ---
        def x_slab(n, suffix=""):
            xs = xp.tile([128, NDC, 2, TOKG], F8, tag="x", name=f"x{suffix}")
            for dq in range(2):
                nc.sync.dma_start(out=xs[:, dq * 16:(dq + 1) * 16, :, :],
                                  in_=x8_d[n, :, dq * 16:(dq + 1) * 16, :, :])
            return xs

        # group 0 pieces in consumption order; wl[e] is only needed by
        # term 3 at the end of phase e, so it trails wh by one phase
        x0 = xp.tile([128, NDC, 2, TOKG], F8, tag="x", name="x0")
        for dq in range(4):
            nc.sync.dma_start(out=x0[:, dq * 8:(dq + 1) * 8, :, :],
                              in_=x8_d[0, :, dq * 8:(dq + 1) * 8, :, :])
            if dq == 0:
                nc.sync.dma_start(out=wh[:, 0, 0:16, :],
                                  in_=wh_d[0, :, 0:16, :])
            elif dq == 1:
                nc.sync.dma_start(out=wh[:, 0, 16:32, :],
                                  in_=wh_d[0, :, 16:32, :])
        nc.sync.dma_start(out=wl[:, 0, :, :], in_=wl_d[0])
        for e in range(1, NE):
            nc.sync.dma_start(out=wh[:, e, :, :], in_=wh_d[e])
            nc.sync.dma_start(out=wl[:, e - 1 + 1, :, :], in_=wl_d[e])
        xpre = {0: x0, 1: x_slab(1, "1")}
        nc.sync.dma_start(out=cos_sb, in_=cos_d[:])
        nc.sync.dma_start(out=sin_sb, in_=sin_d[:])
        nc.sync.dma_start(out=mask_sb,
                          in_=masks_d[:].rearrange("n p q -> p n q"))
        xpre[2] = x_slab(2, "2")
        nc.sync.dma_start(out=woh, in_=woh_d[:])
        nc.sync.dma_start(out=wol, in_=wol_d[:])
        xpre[3] = x_slab(3, "3")

        xsb_cur = xpre[0]
        prevw = None        # (window_n, wtiles) with pending pieces
        for n in range(NG):
            # prefetch next x slab (first few were issued in the prologue)
            if n + 1 < NG and n + 1 not in xpre:
                xpre[n + 1] = x_slab(n + 1, str(n + 1))
            xsb_nxt = xpre.get(n + 1)

            last = n == NG - 1
            # last group: k and q first with inline rope, so the final
            # window's exp chains start as early as possible
            e_order = (4, 0, 1, 2, 3, 5) if last else (0, 1, 2, 3, 4, 5)
            wtiles = {}
            for step, e in enumerate(e_order):
                p1_phase(n, e, xsb_cur)
                if last and e != 5:
                    rope_one(n, e)
                # window n-1 pieces between P1 eviction units
                if prevw is not None and step < 6:
                    wn, wout = prevw
                    if step < 4:
                        window_head(wn, step, wout)
                    elif step == 4:
                        window_p3(wn, 0, wout)
                    else:
                        window_p3(wn, 1, wout)
            prevw = None
            xsb_cur = xsb_nxt

            if not last:
                # RoPE in place for this group (k first: QK needs it)
                rope_one(n, 4)
                for hh in range(HL):
                    rope_one(n, hh)
            prevw = (n, wtiles)

        # final window: heads + P3, nothing left to interleave
        wn, wout = prevw
        for hh in range(HL):
            window_head(wn, hh, wout, tail=True)
        window_p3(wn, 0, wout)
        window_p3(wn, 1, wout)

    nc.finalize()
    return nc, nmask


_CACHE = {}


def _get_nc(window: int):
    if window not in _CACHE:
        _CACHE[window] = build_nc(window)
    return _CACHE[window]


LAST_RESULTS = None


def _hilo(a32):
    hi = a32.astype(E4NP)
    lo = (a32 - hi.astype(np.float32)).astype(E4NP)
    return hi, lo


def kernel(x, w_qkv, w_o, window_size, _trace=False):
    window = int(window_size)
    nc, nmask = _get_nc(window)
    _, keys = _mask_plan(window)
    masks = _build_masks(window, keys)

    xT = np.ascontiguousarray(x.reshape(TOK, D).T).astype(np.float32)
    xh, xl = _hilo(xT)
    # [NG, 128, NDC, 2, TOKG]: hi/lo interleaved per dc chunk
    xh_r = xh.reshape(NDC, 128, NG, TOKG)
    xl_r = xl.reshape(NDC, 128, NG, TOKG)
    x8 = np.ascontiguousarray(
        np.stack([xh_r, xl_r], axis=3).transpose(2, 1, 0, 3, 4))

    inv = 1.0 / (THETA ** (np.arange(0, HD, 2, dtype=np.float64) / HD))
    freqs = np.arange(T, dtype=np.float64)[:, None] * inv[None, :]  # [T, 64]
    cosH = np.repeat(np.cos(freqs).T, 2, axis=0).astype(F16NP)  # [128, T]
    sign = np.where(np.arange(HD) % 2 == 0, -1.0, 1.0)[:, None]
    sinH = (np.repeat(np.sin(freqs).T, 2, axis=0) * sign).astype(F16NP)

    in_maps = []
    for c in range(NCORES):
        wq = w_qkv[QROWS * c:QROWS * (c + 1)]
        wk = w_qkv[H * HD + HD * c: H * HD + HD * (c + 1)]
        wv = w_qkv[H * HD + G * HD + HD * c: H * HD + G * HD + HD * (c + 1)]
        wqkvT = np.concatenate([wq, wk, wv], axis=0).T * WSC  # [D, E] f32
        # e-major pack: [e_chunk, partition(=dc inner 128 rows), dc, 128]
        wE = np.ascontiguousarray(
            wqkvT.reshape(NDC, 128, NE, 128).transpose(2, 1, 0, 3)
        ).astype(np.float32)
        wh_, wl_ = _hilo(wE)
        woT = np.ascontiguousarray(
            (w_o[:, QROWS * c:QROWS * (c + 1)] * WSC).T.reshape(HL, 128, D)
            .transpose(1, 0, 2)).astype(np.float32)  # [128, HL, D]
        woh_, wol_ = _hilo(woT)
        in_maps.append({
            "x8": x8, "wh": wh_, "wl": wl_,
            "woh": woh_, "wol": wol_,
            "cosH": cosH, "sinH": sinH, "masks": masks.astype(F16NP),
        })

    from concourse.bass_utils import run_bass_kernel_spmd
    res = run_bass_kernel_spmd(nc, in_maps, core_ids=list(range(NCORES)),
                               trace=_trace)
    global LAST_RESULTS
    LAST_RESULTS = res
    acc = res.results[0]["out"].astype(np.float32)
    for c in range(1, NCORES):
        acc = acc + res.results[c]["out"].astype(np.float32)
    return acc.reshape(B, T, D)


# revision 38
# speedup vs baseline: 1.3819x; 1.0032x over previous
"""Trainium2 Bass kernel: fused QKV + RoPE + causal/windowed GQA attention + output proj.

Sharding: tensor-parallel by head across 8 cores. Core c owns Q-heads
4c..4c+3 and KV-group c, plus the 512 w_o columns for those heads. Each
core computes a full-shape fp16 partial of the final output; the host
sums the 8 partials. No device collectives.

Single fused pipeline: after P1 finishes the 256-token group
n = b*8 + g2, that group's K/V rows are final, so window (b, g2) of
attention is emitted staggered into the next group's P1 stream - P2/P3
instructions fill PE gaps inside P1's stream and vice versa.

The two big projections (P1, P3) run as fp8e4m3 DoubleRow matmuls
(0.5 PE cycles/row, 256-deep contraction) using an exact hi/lo split:
  a*b ~= a_hi*b_hi + a_hi*b_lo + a_lo*b_hi,   v_hi = fp8(v),
  v_lo = fp8(v - v_hi)  (weights pre-scaled x64 on host so residuals
  clear the fp8 subnormal floor; the x64 / x16 factors are folded into
  eviction scales for free).
3 terms x 0.5 cycles x half the instructions = 0.75x the PE time of
fp16 at ~0.2% error. Attention (QK/PV, contraction 128) stays fp16:
  P2: per (b, g2, head): ST[k, q] pair-tiles -> exp pairs into a window
      est buffer [128, 256, nvis]; diagonal chunks masked by DVE mult;
      denominator = staged DVE chunk-reduces + Pool partition_all_reduce
      (no rowsum matmuls); PV accumulates o_ps; DVE normalizes (x16)
      and splits the result into fp8 hi/lo for P3.
"""

import math
import sys
from contextlib import ExitStack

import numpy as np

sys.path.insert(0, "/opt/trn_rl_repo")

import ml_dtypes

F16NP = np.float16
E4NP = ml_dtypes.float8_e4m3

import concourse.bass as bass
import concourse.mybir as mybir
import concourse.tile as tile
from concourse import bacc
from concourse import bass_isa

F32 = mybir.dt.float32
F16 = mybir.dt.float16
F8 = mybir.dt.float8e4
DR = mybir.MatmulPerfMode.DoubleRow

B, T, D = 2, 2048, 4096
H, G, HD = 32, 8, 128
THETA = 10000.0
NCORES = 8
HL = H // NCORES            # 4 local q heads
TOK = B * T                 # 4096
QROWS = HL * HD             # 512 local q rows
E = QROWS + 2 * HD          # 768 local qkv rows
SCALE = 1.0 / math.sqrt(HD)
WSC = 64.0                  # host pre-scale on w_qkv / w_o (fp8 subnormals)
ASC = 16.0                  # device pre-scale on attn output before fp8 split

TOKG = 256                  # P1 token-group width == P2 query-group width
NG = TOK // TOKG            # 16 groups; group n = (b= n//8, g2= n%8)
NGB = T // TOKG             # 8 groups per batch
NDC = D // 128              # 32 contraction chunks
NDP = NDC // 2              # 16 DoubleRow chunk pairs
NE = E // 128               # 6 qkv row chunks (4 q, 1 k, 1 v)
NKC = T // 128              # 16 key chunks per batch


def _mask_plan(window: int):
    """Per (g2, kc): 'skip', 'full', or mask-key (i-j offset based).

    Chunks are always computed full-width (256 queries); masked chunks
    multiply by a {0,1} mask afterward, so the est buffer holds exact
    zeros outside the visible region.
    """
    plan = {}
    keys = {}
    for g in range(NGB):
        for kc in range(NKC):
            i_min, i_max = TOKG * g, TOKG * g + TOKG - 1
            j_min, j_max = 128 * kc, 128 * kc + 127
            if j_min > i_max or (i_min - j_max) >= window:
                plan[(g, kc)] = ("skip", None)
            elif j_max <= i_min and (i_max - j_min) < window:
                plan[(g, kc)] = ("full", None)
            else:
                key = TOKG * g - 128 * kc
                if key not in keys:
                    keys[key] = len(keys)
                plan[(g, kc)] = ("mask", keys[key])
    return plan, keys


def _build_masks(window: int, keys: dict) -> np.ndarray:
    n = max(1, len(keys))
    m = np.zeros((n, 128, TOKG), dtype=np.float32)
    for key, idx in keys.items():
        qq = np.arange(TOKG)[None, :]
        kk = np.arange(128)[:, None]
        diff = key + qq - kk          # i - j
        vis = (diff >= 0) & (diff < window)
        m[idx] = np.where(vis, 1.0, 0.0)
    return m


PAIRSWAP = [i ^ 1 for i in range(32)]


def _rope_ops(nc, pool, dst, src, cos_ap, sin_ap):
    """Interleaved-pair RoPE: dst = src*cos + pairswap(src)*signed_sin.

    cos_ap rows (2i, 2i+1) hold cos_i; sin_ap rows hold (-sin_i, +sin_i).
    src may alias dst (in-place).
    """
    W = dst.shape[-1]
    sw = pool.tile([128, W], F16, tag="rope_sw", name="rope_sw")
    tmp = pool.tile([128, W], F16, tag="rope_tmp", name="rope_tmp")
    qc = pool.tile([128, W], F16, tag="rope_qc", name="rope_qc")
    mult = mybir.AluOpType.mult
    nc.vector.stream_shuffle(sw, src, PAIRSWAP)
    nc.vector.tensor_tensor(tmp, sw, sin_ap, mult)
    nc.vector.tensor_tensor(qc, src, cos_ap, mult)
    nc.vector.tensor_tensor(dst, qc, tmp, mybir.AluOpType.add)


def build_nc(window: int):
    plan, keys = _mask_plan(window)
    nmask = max(1, len(keys))

    nc = bacc.Bacc()
    # hi/lo planes interleaved per dc chunk so DMA runs stay >= 512B
    x8_d = nc.dram_tensor("x8", [NG, 128, NDC, 2, TOKG], F8,
                          kind="ExternalInput")
    # host-packed e-major: [e_chunk, partition, dc, 128], pre-scaled x64
    wh_d = nc.dram_tensor("wh", [NE, 128, NDC, 128], F8, kind="ExternalInput")
    wl_d = nc.dram_tensor("wl", [NE, 128, NDC, 128], F8, kind="ExternalInput")
    # [partition, local_head_chunk, D], pre-scaled x64
    woh_d = nc.dram_tensor("woh", [128, HL, D], F8, kind="ExternalInput")
    wol_d = nc.dram_tensor("wol", [128, HL, D], F8, kind="ExternalInput")
    cos_d = nc.dram_tensor("cosH", [128, T], F16, kind="ExternalInput")
    sin_d = nc.dram_tensor("sinH", [128, T], F16, kind="ExternalInput")
    masks_d = nc.dram_tensor("masks", [nmask, 128, TOKG], F16, kind="ExternalInput")
    out_d = nc.dram_tensor("out", [TOK, D], F16, kind="ExternalOutput")

    with ExitStack() as octx:
        tc = octx.enter_context(tile.TileContext(nc))
        # persistent SBUF
        pers = octx.enter_context(tc.tile_pool(name="pers", bufs=1))
        wh = pers.tile([128, NE, NDC, 128], F8, name="wh")
        wl = pers.tile([128, NE, NDC, 128], F8, name="wl")
        qkvT = [pers.tile([128, TOK], F16, tag=f"qkv{e}", name=f"qkv{e}")
                for e in range(5)]               # 4 q heads + k
        ksb = qkvT[HL]
        vsb = pers.tile([128, TOK // 128, 128], F16, name="vsb")
        cos_sb = pers.tile([128, T], F16, name="cos_sb")
        sin_sb = pers.tile([128, T], F16, name="sin_sb")
        mask_sb = pers.tile([128, nmask, TOKG], F16, name="mask_sb")
        woh = pers.tile([128, HL, D], F8, name="woh")
        wol = pers.tile([128, HL, D], F8, name="wol")

        # working pools
        xp = octx.enter_context(tc.tile_pool(name="xp", bufs=2))
        ep = octx.enter_context(tc.tile_pool(name="ep", bufs=2))
        rp = octx.enter_context(tc.tile_pool(name="rp", bufs=2))
        op = octx.enter_context(tc.tile_pool(name="op", bufs=2))
        ps_p1 = octx.enter_context(tc.tile_pool(name="ps1", bufs=2, space="PSUM"))
        ps_st = octx.enter_context(tc.tile_pool(name="ps_st", bufs=2, space="PSUM"))
        ps_o = octx.enter_context(tc.tile_pool(name="ps_o", bufs=2, space="PSUM"))
        ps_p3 = octx.enter_context(tc.tile_pool(name="ps3", bufs=2, space="PSUM"))

        def window_head(n, hh, wtiles, tail=False, steal=None):
            """One head's QK->exp->PV->denominator->normalize chain.

            tail=True (final window, P1 finished): steal idle ps_p1 banks
            as extra score buffers and stage the denominator reduce in
            more, smaller pieces (the tail has no P1 to hide latency).
            """
            b, g2 = divmod(n, NGB)
            t0 = n * TOKG
            vis = [(kc, plan[(g2, kc)]) for kc in range(NKC)
                   if plan[(g2, kc)][0] != "skip"]
            nvis = len(vis)
            if hh == 0:
                wtiles["h"] = op.tile([128, HL, TOKG], F8, tag="owh",
                                      name="owh")
                wtiles["l"] = op.tile([128, HL, TOKG], F8, tag="owl",
                                      name="owl")
            esw = ep.tile([128, TOKG, nvis], F16, tag="esw", name="esw",
                          padded_shape=[128, TOKG, NKC])
            o_ps = ps_o.tile([128, TOKG], F32, tag="o", name="o_ps")
            qtile = qkvT[hh][:, t0:t0 + TOKG]
            racc1 = None
            if steal is None:
                steal = tail
            mids_at = ([nvis // 4 * 2, nvis - 2] if tail and nvis > 6
                       else [nvis - 2])
            # QK + exp in pairs sharing one PSUM bank
            i = 0
            while i < nvis:
                npair = min(2, nvis - i)
                stp = ps_p1 if (steal and (i // 2) % 2 == 1) else ps_st
                st = stp.tile([128, 2, TOKG], F32,
                              tag="p1" if stp is ps_p1 else "st",
                              name="st")
                for j in range(npair):
                    kc = vis[i + j][0]
                    nc.tensor.matmul(
                        st[:, j, :],
                        lhsT=ksb[:, b * T + kc * 128:b * T + (kc + 1) * 128],
                        rhs=qtile,
                        start=True, stop=True)
                nc.scalar.activation(
                    esw[:, :, i:i + npair].rearrange("p q s -> p s q"),
                    st[:, 0:npair, :],
                    mybir.ActivationFunctionType.Exp)
                # mask chunks in this pair (diagonal / window edges)
                for j in range(npair):
                    kind, mid = vis[i + j][1]
                    if kind == "mask":
                        sl = esw[:, :, i + j]
                        nc.vector.tensor_tensor(
                            sl, sl, mask_sb[:, mid, :],
                            mybir.AluOpType.mult)
                i += npair
                # staged partial denominators keep the post-exp chain short
                if i in mids_at and nvis > 2:
                    part = rp.tile([128, TOKG], F16, tag="racc1",
                                   name="racc1")
                    lo = 0 if racc1 is None else racc1[1]
                    with nc.allow_low_precision(
                            "est chunk-reduce; denominator finished in f32"):
                        nc.vector.tensor_reduce(
                            part, esw[:, :, lo:i], mybir.AxisListType.X,
                            mybir.AluOpType.add)
                    if racc1 is None:
                        racc1 = (part, i)
                    else:
                        comb = rp.tile([128, TOKG], F16, tag="rcomb",
                                       name="racc1c")
                        nc.vector.tensor_tensor(
                            comb, racc1[0], part, mybir.AluOpType.add)
                        racc1 = (comb, i)
            # PV accumulation (trim pure-causal zero prefix columns)
            for idx, (kc, (kind, mid)) in enumerate(vis):
                qlo = max(0, 128 * kc - TOKG * g2)
                nc.tensor.matmul(
                    o_ps[:, qlo:TOKG],
                    lhsT=vsb[:, b * NKC + kc, :],
                    rhs=esw[:, qlo:TOKG, idx],
                    start=(idx == 0), stop=(idx == nvis - 1))
            # finish the denominator (short tail after the last exp)
            acc = rp.tile([128, TOKG], F32, tag="acc", name="acc")
            with nc.allow_low_precision(
                    "fp16 partial + f32 combine for denominator"):
                if racc1 is not None:
                    racc2 = rp.tile([128, TOKG], F16, tag="racc2",
                                    name="racc2")
                    nc.vector.tensor_reduce(
                        racc2, esw[:, :, racc1[1]:nvis],
                        mybir.AxisListType.X, mybir.AluOpType.add)
                    nc.vector.tensor_tensor(
                        acc, racc1[0], racc2, mybir.AluOpType.add)
                else:
                    nc.vector.tensor_reduce(
                        acc, esw, mybir.AxisListType.X,
                        mybir.AluOpType.add)
            # partition reduce of the accumulated denominator (Pool)
            rsum = rp.tile([128, TOKG], F32, tag="rsum", name="rsum")
            nc.gpsimd.partition_all_reduce(
                rsum, acc, channels=128, reduce_op=bass_isa.ReduceOp.add)
            rrec = rp.tile([128, TOKG], F32, tag="rrec", name="rrec")
            nc.vector.reciprocal(rrec, rsum)
            # normalize (xASC) and split fp8 hi/lo for the DoubleRow P3
            at16 = rp.tile([128, TOKG], F16, tag="at16", name="at16")
            nc.vector.scalar_tensor_tensor(
                at16, o_ps, ASC, rrec,
                mybir.AluOpType.mult, mybir.AluOpType.mult)
            nc.scalar.copy(wtiles["h"][:, hh, :], at16)
            nc.vector.tensor_tensor(
                wtiles["l"][:, hh, :], at16, wtiles["h"][:, hh, :],
                mybir.AluOpType.subtract)

        def window_p3(n, tloc, wtiles):
            """Output projection for one 128-token chunk of window n."""
            t0 = n * TOKG
            tch = t0 // 128 + tloc
            owh_t, owl_t = wtiles["h"], wtiles["l"]
            tsl = slice(tloc * 128, (tloc + 1) * 128)
            for et in range(D // 512):
                ps = ps_p3.tile([128, 512], F32, tag="p3", name="p3ps")
                esl = slice(et * 512, (et + 1) * 512)
                for dcp in range(HL // 2):
                    hsl = slice(2 * dcp, 2 * dcp + 2)
                    for tnum, (lo, ro) in enumerate(
                            ((owh_t, woh), (owh_t, wol), (owl_t, woh))):
                        nc.tensor.matmul(
                            ps,
                            lhsT=lo[:, hsl, tsl],
                            rhs=ro[:, hsl, esl],
                            start=(dcp == 0 and tnum == 0),
                            stop=(dcp == HL // 2 - 1 and tnum == 2),
                            perf_mode=DR)
                pan = op.tile([128, 512], F16, tag="pan", name="pan",
                              bufs=5)
                if et % 3 != 2:
                    nc.scalar.mul(pan, ps, 1.0 / (WSC * ASC))
                else:
                    nc.vector.tensor_scalar_mul(pan, ps, 1.0 / (WSC * ASC))
                nc.sync.dma_start(
                    out=out_d[tch * 128:(tch + 1) * 128, esl],
                    in_=pan)

        def p1_phase(n, e, xsc):
            """One eviction-unit of P1 as 3-term hi/lo DoubleRow fp8:
            e in 0..4 -> qkv row chunk; e == 5 -> both v token chunks."""
            t0 = n * TOKG
            if e < 5:
                ps = ps_p1.tile([128, TOKG], F32, tag="p1", name="p1ps")
                for tnum, (lw, xs) in enumerate(
                        ((wh, 0), (wh, 1), (wl, 0))):
                    for dp in range(NDP):
                        nc.tensor.matmul(
                            ps,
                            lhsT=lw[:, e, 2 * dp:2 * dp + 2, :],
                            rhs=xsc[:, 2 * dp:2 * dp + 2, xs, :],
                            start=(tnum == 0 and dp == 0),
                            stop=(tnum == 2 and dp == NDP - 1),
                            perf_mode=DR)
                # fold softmax 1/sqrt(HD) and the 1/WSC w-prescale into
                # the eviction
                nc.scalar.mul(qkvT[e][:, t0:t0 + TOKG], ps,
                              (SCALE if e < HL else 1.0) / WSC)
            else:
                for tch in range(TOKG // 128):  # v in [tok, hd] orientation
                    vp = ps_p1.tile([128, 128], F32, tag="p1", name="vps")
                    ts = slice(tch * 128, (tch + 1) * 128)
                    for tnum, (xs, rw) in enumerate(
                            ((0, wh), (0, wl), (1, wh))):
                        for dp in range(NDP):
                            nc.tensor.matmul(
                                vp,
                                lhsT=xsc[:, 2 * dp:2 * dp + 2, xs, ts],
                                rhs=rw[:, 5, 2 * dp:2 * dp + 2, :],
                                start=(tnum == 0 and dp == 0),
                                stop=(tnum == 2 and dp == NDP - 1),
                                perf_mode=DR)
                    nc.scalar.mul(vsb[:, t0 // 128 + tch, :], vp, 1.0 / WSC)

        def rope_one(n, e):
            """In-place RoPE of group n's row-chunk e (0..3 q heads, 4 k)."""
            g2 = n % NGB
            t0 = n * TOKG
            cs = cos_sb[:, g2 * TOKG:(g2 + 1) * TOKG]
            sn = sin_sb[:, g2 * TOKG:(g2 + 1) * TOKG]
            _rope_ops(nc, op, qkvT[e][:, t0:t0 + TOKG],
                      qkvT[e][:, t0:t0 + TOKG], cs, sn)

        # --- prologue DMAs, just-in-time order (transfers serialize) ---
        def x_slab(n, suffix=""):
            xs = xp.tile([128, NDC, 2, TOKG], F8, tag="x", name=f"x{suffix}")
            for dq in range(2):
                nc.sync.dma_start(out=xs[:, dq * 16:(dq + 1) * 16, :, :],
                                  in_=x8_d[n, :, dq * 16:(dq + 1) * 16, :, :])
            return xs

        # group 0 pieces in consumption order; wl[e] is only needed by
        # term 3 at the end of phase e, so it trails wh by one phase
        x0 = xp.tile([128, NDC, 2, TOKG], F8, tag="x", name="x0")
        for dq in range(4):
            nc.sync.dma_start(out=x0[:, dq * 8:(dq + 1) * 8, :, :],
                              in_=x8_d[0, :, dq * 8:(dq + 1) * 8, :, :])
            if dq == 0:
                nc.sync.dma_start(out=wh[:, 0, 0:16, :],
                                  in_=wh_d[0, :, 0:16, :])
            elif dq == 1:
                nc.sync.dma_start(out=wh[:, 0, 16:32, :],
                                  in_=wh_d[0, :, 16:32, :])
        nc.sync.dma_start(out=wl[:, 0, :, :], in_=wl_d[0])
        for e in range(1, NE):
            nc.sync.dma_start(out=wh[:, e, :, :], in_=wh_d[e])
            nc.sync.dma_start(out=wl[:, e - 1 + 1, :, :], in_=wl_d[e])
        xpre = {0: x0, 1: x_slab(1, "1")}
        nc.sync.dma_start(out=cos_sb, in_=cos_d[:])
        nc.sync.dma_start(out=sin_sb, in_=sin_d[:])
        nc.sync.dma_start(out=mask_sb,
                          in_=masks_d[:].rearrange("n p q -> p n q"))
        xpre[2] = x_slab(2, "2")
        nc.sync.dma_start(out=woh, in_=woh_d[:])
        nc.sync.dma_start(out=wol, in_=wol_d[:])
        xpre[3] = x_slab(3, "3")

        xsb_cur = xpre[0]
        prevw = None        # (window_n, wtiles) with pending pieces
        for n in range(NG):
            # prefetch next x slab (first few were issued in the prologue)
            if n + 1 < NG and n + 1 not in xpre:
                xpre[n + 1] = x_slab(n + 1, str(n + 1))
            xsb_nxt = xpre.get(n + 1)

            last = n == NG - 1
            endgame = n >= NG - 2
            # last two groups: k and q first with inline rope, so the
            # final windows' exp chains start as early as possible; the
            # very last window's heads are double-staggered into its own
            # group's P1 stream
            if last:
                e_order = (4, 5, 0, 1, 2, 3)   # v early: W15 PV needs it
            elif endgame:
                e_order = (4, 0, 1, 2, 3, 5)
            else:
                e_order = (0, 1, 2, 3, 4, 5)
            wtiles = {}
            for step, e in enumerate(e_order):
                p1_phase(n, e, xsb_cur)
                if endgame and e != 5:
                    rope_one(n, e)
                # window n-1 pieces between P1 eviction units
                if prevw is not None and step < 6:
                    wn, wout = prevw
                    if step < 4:
                        window_head(wn, step, wout)
                    elif step == 4:
                        window_p3(wn, 0, wout)
                    else:
                        window_p3(wn, 1, wout)
                if last and step >= 2:
                    window_head(n, step - 2, wtiles, tail=True, steal=False)
            prevw = None
            xsb_cur = xsb_nxt

            if not endgame:
                # RoPE in place for this group (k first: QK needs it)
                rope_one(n, 4)
                for hh in range(HL):
                    rope_one(n, hh)
            prevw = (n, wtiles)

        # final window: only its output projection remains
        wn, wout = prevw
        window_p3(wn, 0, wout)
        window_p3(wn, 1, wout)

    nc.finalize()
    return nc, nmask


_CACHE = {}


def _get_nc(window: int):
    if window not in _CACHE:
        _CACHE[window] = build_nc(window)
    return _CACHE[window]


LAST_RESULTS = None


def _hilo(a32):
    hi = a32.astype(E4NP)
    lo = (a32 - hi.astype(np.float32)).astype(E4NP)
    return hi, lo


def kernel(x, w_qkv, w_o, window_size, _trace=False):
    window = int(window_size)
    nc, nmask = _get_nc(window)
    _, keys = _mask_plan(window)
    masks = _build_masks(window, keys)

    xT = np.ascontiguousarray(x.reshape(TOK, D).T).astype(np.float32)
    xh, xl = _hilo(xT)
    # [NG, 128, NDC, 2, TOKG]: hi/lo interleaved per dc chunk
    xh_r = xh.reshape(NDC, 128, NG, TOKG)
    xl_r = xl.reshape(NDC, 128, NG, TOKG)
    x8 = np.ascontiguousarray(
        np.stack([xh_r, xl_r], axis=3).transpose(2, 1, 0, 3, 4))

    inv = 1.0 / (THETA ** (np.arange(0, HD, 2, dtype=np.float64) / HD))
    freqs = np.arange(T, dtype=np.float64)[:, None] * inv[None, :]  # [T, 64]
    cosH = np.repeat(np.cos(freqs).T, 2, axis=0).astype(F16NP)  # [128, T]
    sign = np.where(np.arange(HD) % 2 == 0, -1.0, 1.0)[:, None]
    sinH = (np.repeat(np.sin(freqs).T, 2, axis=0) * sign).astype(F16NP)

    in_maps = []
    for c in range(NCORES):
        wq = w_qkv[QROWS * c:QROWS * (c + 1)]
        wk = w_qkv[H * HD + HD * c: H * HD + HD * (c + 1)]
        wv = w_qkv[H * HD + G * HD + HD * c: H * HD + G * HD + HD * (c + 1)]
        wqkvT = np.concatenate([wq, wk, wv], axis=0).T * WSC  # [D, E] f32
        # e-major pack: [e_chunk, partition(=dc inner 128 rows), dc, 128]
        wE = np.ascontiguousarray(
            wqkvT.reshape(NDC, 128, NE, 128).transpose(2, 1, 0, 3)
        ).astype(np.float32)
        wh_, wl_ = _hilo(wE)
        woT = np.ascontiguousarray(
            (w_o[:, QROWS * c:QROWS * (c + 1)] * WSC).T.reshape(HL, 128, D)
            .transpose(1, 0, 2)).astype(np.float32)  # [128, HL, D]
        woh_, wol_ = _hilo(woT)
        in_maps.append({
            "x8": x8, "wh": wh_, "wl": wl_,
            "woh": woh_, "wol": wol_,
            "cosH": cosH, "sinH": sinH, "masks": masks.astype(F16NP),
        })

    from concourse.bass_utils import run_bass_kernel_spmd
    res = run_bass_kernel_spmd(nc, in_maps, core_ids=list(range(NCORES)),
                               trace=_trace)
    global LAST_RESULTS
    LAST_RESULTS = res
    acc = res.results[0]["out"].astype(np.float32)
    for c in range(1, NCORES):
        acc = acc + res.results[c]["out"].astype(np.float32)
    return acc.reshape(B, T, D)
